# revision 27
# baseline (speedup 1.0000x reference)
"""Trainium2 Bass kernel for nn_Block_38517266710836.

reference pipeline: channel mixer -> STFT (hann 2048, hop 1024) -> per-frame
recurrence out[f] = (spec[f] + out[f-1]) * transfer -> iSTFT (hann synthesis)
-> overlap-add -> gain -> tanh.

Sharding: 8 cores, data-parallel over (batch, channel-half): core c handles
batch c//2, mixed channels [32*(c%2), +32). Each core receives its batch's
full 64-channel input (the mixer contracts channels) and writes 32 rows.

Pipelined single-pass program per core: mixer chunks, forward-DFT frame
batches, the DVE recurrence scan, and the inverse/overlap-add phase are
interleaved so the scan and evictions hide under PE matmul work.  Forward
evictions and corner-turn copies run on ScalarE (DVE is reserved for the
scan), weights stream on the gpsimd DMA queue, x/y on the sync queue, and
PSUM pools are shared across phases to fit the 8-bank budget.
"""

import numpy as np

WINDOW = 2048
STEP = 1024
CPD = 64
BATCH = 4
TIME = 65536
FRAMES = 64
NJ = 16              # per-frame time chunks (fwd contraction blocks)
NM = 16              # spectral slot chunks
DSH = 32             # mixed channels per core
GCH = TIME // 128    # 512 global 128-sample chunks
GPAD = GCH + 16      # + zero pad (frame 63 reaches t=66560; extra width so
                     # the forward rhs slice [base, base+2048) stays in-bounds)
FC = 4               # frame chunks for the scan layout
FW = 16              # frames per chunk
CB = 17              # chain block: 1 inject/reset col + 16 frame cols
SPECW = NM * DSH * CB  # 8704 free cols per fc block


def _hann(n):
    return (0.5 - 0.5 * np.cos(2.0 * np.pi * np.arange(n) / n)).astype(np.float64)


def _slot_tables():
    """slot s in [0,2048): s<1024 -> Re[k=s]; s==1024 -> Re[1024] (parked in
    Im[0]'s slot, since Im[0] is identically 0); s>1024 -> Im[k=s-1024]."""
    k_of_slot = np.zeros(2048, np.int64)
    is_im = np.zeros(2048, np.bool_)
    for s in range(2048):
        if s < 1024:
            k_of_slot[s] = s
        elif s == 1024:
            k_of_slot[s] = 1024
        else:
            k_of_slot[s] = s - 1024
            is_im[s] = True
    return k_of_slot, is_im


def build_fwd_weights():
    """[2048 n, 2048 slots]: windowed rfft of one frame, slot layout."""
    n = np.arange(WINDOW, dtype=np.float64)
    w = _hann(WINDOW)
    k_of_slot, is_im = _slot_tables()
    ang = 2.0 * np.pi * np.outer(n, k_of_slot.astype(np.float64)) / WINDOW
    W = np.where(is_im[None, :], -np.sin(ang), np.cos(ang))
    W *= w[:, None]
    return W


def build_inv_weights(gain):
    """[2048 slots, 2048 n]: gain * hann * irfft from slot layout."""
    n = np.arange(WINDOW, dtype=np.float64)
    w = _hann(WINDOW)
    k_of_slot, is_im = _slot_tables()
    ang = 2.0 * np.pi * np.outer(k_of_slot.astype(np.float64), n) / WINDOW
    k = k_of_slot
    re_coef = (2.0 - (k == 0) - (k == 1024))[:, None] / WINDOW * np.cos(ang)
    im_coef = -2.0 / WINDOW * np.sin(ang)
    W = np.where(is_im[:, None], im_coef, re_coef)
    W[1024, :] = np.cos(np.pi * n) / WINDOW
    W *= (gain * w)[None, :]
    return W


def _slot_tables_r4():
    """family-major slot layout: slot' = s*512 + local; family s holds
    k = s, s+4, ... <= 1024(ish), (re, im) interleaved k-major."""
    karr = np.zeros(2048, np.int64)
    isim = np.zeros(2048, np.bool_)
    pos = 0
    for s in range(4):
        for k in range(s, 1025, 4):
            karr[pos] = k; isim[pos] = False; pos += 1
            if k not in (0, 1024):
                karr[pos] = k; isim[pos] = True; pos += 1
    assert pos == 2048
    return karr, isim


_R4_PLANES = {0: [0], 1: [2, 3], 2: [1], 3: [2, 3]}  # m//4 -> plane list


def _build_wfam():
    """family -> list of (plane, [512 q, 512 r]) weight matrices.
    planes: 0=y0, 1=y2, 2=m0, 3=m1."""
    q = np.arange(512, dtype=np.float64)
    karr, isim = _slot_tables_r4()
    fams = {}
    for s in range(4):
        kv = karr[s * 512:(s + 1) * 512].astype(np.float64)
        iv = isim[s * 512:(s + 1) * 512]
        ang = 2.0 * np.pi * np.outer(q, kv) / WINDOW
        c, sn = np.cos(ang), np.sin(ang)
        if s == 0:
            fams[s] = [(0, np.where(iv[None, :], -sn, c))]
        elif s == 2:
            fams[s] = [(1, np.where(iv[None, :], -sn, c))]
        elif s == 1:
            fams[s] = [(2, np.where(iv[None, :], -sn, c)),
                       (3, np.where(iv[None, :], -c, -sn))]
        else:
            fams[s] = [(2, np.where(iv[None, :], -sn, c)),
                       (3, np.where(iv[None, :], c, sn))]
    return fams


def build_wf4():
    """[128, 96*128] SBUF-ready block layout matching the device MM loop:
    for qp, mi: m=2qp+mi -> (s=m//4, subm=m%4): for plane, for qc: block
    = Wfam[s][plane][qc*128:+128, subm*128:+128]."""
    fams = _build_wfam()
    blocks = []
    for qp in range(8):
        for mi in range(2):
            m = qp * 2 + mi
            s, subm = m // 4, m % 4
            for pl, Wm in fams[s]:
                for qc in range(4):
                    blocks.append(Wm[qc * 128:(qc + 1) * 128,
                                     subm * 128:(subm + 1) * 128])
    return np.concatenate(blocks, axis=1)  # [128, 96*128]


def build_wcol():
    """[128, 16] per-partition window scalars: col j*4+qc = w[qc*128+p+512j]."""
    w = _hann(WINDOW)
    out = np.zeros((128, 16), np.float64)
    for j in range(4):
        for qc in range(4):
            out[:, j * 4 + qc] = w[qc * 128 + np.arange(128) + 512 * j]
    return out


def build_t_slots(transfer):
    karr, _ = _slot_tables_r4()
    return np.asarray(transfer, np.float64)[:, karr]  # [ch, 2048]


_INV_PLANES = [(0, False), (1, False), (1, True), (2, False), (3, False), (3, True)]
# zb plane order: 0=zre0, 1=zre1, 2=zim1, 3=zre2, 4=zre3, 5=zim3


def build_wi4():
    """[128, 96*128] inverse z-plane weights; device order:
    for qc in 4: for pl in 6: for subm in 4."""
    karr, isim = _slot_tables_r4()
    q = np.arange(512, dtype=np.float64)
    Vs = []
    for (s, want_im) in _INV_PLANES:
        kv = karr[s * 512:(s + 1) * 512].astype(np.float64)
        iv = isim[s * 512:(s + 1) * 512]
        coef = (2.0 - (kv == 0) - (kv == 1024)) / WINDOW
        ang = 2.0 * np.pi * np.outer(kv, q) / WINDOW
        c, sn = np.cos(ang), np.sin(ang)
        V = coef[:, None] * (np.where(iv[:, None], c, sn) if want_im
                             else np.where(iv[:, None], -sn, c))
        Vs.append(V)  # [512 slot-reals, 512 q]
    blocks = []
    for qc in range(4):
        for V in Vs:
            for subm in range(4):
                blocks.append(V[subm * 128:(subm + 1) * 128,
                               qc * 128:(qc + 1) * 128])
    return np.concatenate(blocks, axis=1)


def build_inv_weights_perm(gain):
    """[2048 slots', 2048 n]: gain * hann * irfft from the r4 slot layout."""
    n = np.arange(WINDOW, dtype=np.float64)
    w = _hann(WINDOW)
    karr, isim = _slot_tables_r4()
    k = karr.astype(np.float64)
    ang = 2.0 * np.pi * np.outer(k, n) / WINDOW
    re_coef = (2.0 - (karr == 0) - (karr == 1024))[:, None] / WINDOW * np.cos(ang)
    im_coef = -2.0 / WINDOW * np.sin(ang)
    W = np.where(isim[:, None], im_coef, re_coef)
    W *= (gain * w)[None, :]
    return W


def build_fwd_weights_perm():
    """effective [2048 n, 2048 slots'] fwd matrix (validation only)."""
    n = np.arange(WINDOW, dtype=np.float64)
    w = _hann(WINDOW)
    karr, isim = _slot_tables_r4()
    ang = 2.0 * np.pi * np.outer(n, karr.astype(np.float64)) / WINDOW
    W = np.where(isim[None, :], -np.sin(ang), np.cos(ang))
    W *= w[:, None]
    return W


def build_pattern(t_slots_core):
    """T-pattern [128, SPECW]: per (m,d) chain block of CB cols:
    col 0 = 0 (reset: state=(x+state)*0), cols 1..16 = T[slot(m,kf), d]
    for the scan state=(spec+state)*pat; the carry between fc blocks is
    pre-added into the next block's frame-0 column."""
    pat = np.zeros((128, SPECW), np.float64)
    for m in range(NM):
        for d in range(DSH):
            base = (m * DSH + d) * CB
            pat[:, base + 1: base + CB] = \
                t_slots_core[d, m * 128:(m + 1) * 128][:, None]
    return pat


def build_mixw4(mix_half):
    """[128, 64] block-diag lhsT: rows (q,c) q in 2, cols (q',d):
    M[c, d] if q==q' else 0."""
    out = np.zeros((128, 64), np.float64)
    out[0:64, 0:32] = mix_half
    out[64:128, 32:64] = mix_half
    return out


def emulate(x, transfer, mixer_matrix, gain, wdtype=np.float32):
    """Numpy emulation of the device math (offline validation)."""
    b, c, t = x.shape
    Wf = build_fwd_weights_perm().astype(wdtype).astype(np.float64)
    Wi = build_inv_weights_perm(float(np.asarray(gain).ravel()[0])).astype(wdtype).astype(np.float64)
    Ts = build_t_slots(transfer)
    y = np.einsum('bct,cd->bdt', np.asarray(x, np.float64),
                  np.asarray(mixer_matrix, np.float64))
    yp = np.pad(y, ((0, 0), (0, 0), (0, STEP)))
    out = np.zeros((b, c, t), np.float64)
    for bi in range(b):
        frames = np.stack([yp[bi, :, f * STEP: f * STEP + WINDOW]
                           for f in range(FRAMES)], 1)
        spec = frames.astype(wdtype).astype(np.float64) @ Wf
        st = np.zeros((c, 2048))
        outs = np.zeros_like(spec)
        for f in range(FRAMES):
            st = (spec[:, f].astype(wdtype).astype(np.float64) + st) * Ts
            outs[:, f] = st
        aud = outs.astype(wdtype).astype(np.float64) @ Wi
        acc = np.zeros((c, t + STEP))
        for f in range(FRAMES):
            acc[:, f * STEP: f * STEP + WINDOW] += aud[:, f]
        out[bi] = np.tanh(acc[:, :t])
    return out.astype(np.float32)


# ---------------------------------------------------------------------------
# Device program
# ---------------------------------------------------------------------------

_CACHED_NC = None


def _build_program():
    import concourse.bacc as bacc
    import concourse.mybir as mybir
    from concourse import tile
    from contextlib import ExitStack

    f32 = mybir.dt.float32
    bf16 = mybir.dt.bfloat16
    Alu = mybir.AluOpType

    nc = bacc.Bacc("TRN2", target_bir_lowering=False, debug=False, num_devices=8)
    xb = nc.dram_tensor("xb", [CPD, TIME], bf16, kind="ExternalInput").ap()
    mixwd = nc.dram_tensor("mixw4", [128, 64], bf16, kind="ExternalInput").ap()
    eyebd = nc.dram_tensor("eyeb", [128, 128], bf16, kind="ExternalInput").ap()
    wf4 = nc.dram_tensor("wf4", [128, 96 * 128], bf16, kind="ExternalInput").ap()
    wcold = nc.dram_tensor("wcol", [128, 16], f32, kind="ExternalInput").ap()
    wi4d = nc.dram_tensor("wi4", [128, 96 * 128], bf16, kind="ExternalInput").ap()
    wicold = nc.dram_tensor("wicol", [128, 16], f32, kind="ExternalInput").ap()
    patd = nc.dram_tensor("pat", [128, SPECW], bf16, kind="ExternalInput").ap()
    eyed = nc.dram_tensor("eye", [128, 128], f32, kind="ExternalInput").ap()
    yout = nc.dram_tensor("y", [DSH, TIME], f32, kind="ExternalOutput").ap()

    XCH = 4096           # x streamed in 4096-sample chunks
    NXC = TIME // XCH    # 16
    # scan split: DVE takes m-blocks 0..8, Pool 9..15
    SCAN_DVE = 9 * DSH * CB   # 4896 cols
    # x view [nx, q, c, h, f]: t = nx*4096 + h*2048 + q*1024 + f
    xbv = xb.rearrange("c (nx h q f) -> nx q c h f", nx=NXC, h=2, q=2)

    with tile.TileContext(nc) as tc, ExitStack() as ctx:
        persist = ctx.enter_context(tc.tile_pool(name="persist", bufs=1))
        spec = persist.tile([128, FC * SPECW], bf16, tag="spec")
        pat = persist.tile([128, SPECW], bf16, tag="pat")
        mx4 = persist.tile([128, 64], bf16, tag="mx4")
        eyeb = persist.tile([128, 128], bf16, tag="eyeb")
        eye = persist.tile([128, 128], f32, tag="eye")
        wcol = persist.tile([128, 16], f32, tag="wcol")

        xin = ctx.enter_context(tc.tile_pool(name="xin", bufs=3))
        ymp = ctx.enter_context(tc.tile_pool(name="ymp", bufs=3))



        # reset cols (col 0 of every chain) must not contain NaN garbage:
        # (garbage + state) * 0 is 0 only for finite garbage
        nc.vector.memset(
            spec[:].rearrange("p (fcmd c) -> p fcmd c", c=CB)[:, :, 0:1], 0.0)

        # Pool/gpsimd cannot access PSUM (walrus birverifier) — psum
        # evictions go on Act/DVE only
        _TURN_ENG = [nc.scalar, nc.vector]

        def mixer_chunk(xc):
            # xt partitions (q,c), free (h,f): t = xc*4096 + h*2048 + q*1024 + f
            xt = xin.tile([128, 2048], bf16, tag="x", name=f"x{xc}")
            for q in range(2):
                nc.sync.dma_start(
                    out=xt[q * 64:(q + 1) * 64, :].rearrange(
                        "c (h f) -> c h f", h=2),
                    in_=xbv[xc, q])
            pm = ppM.tile([128, 1024], f32, tag="pp", name=f"mix{xc}")
            for h in range(2):
                for hf in range(2):  # psum-bank-sized output halves
                    nc.tensor.matmul(
                        pm[h * 64:(h + 1) * 64,
                           hf * 512:(hf + 1) * 512],
                        mx4[:],
                        xt[:, h * 1024 + hf * 512: h * 1024 + (hf + 1) * 512],
                        start=True, stop=True)
            # ym[(h,q,d), f]
            ym = ymp.tile([128, 1024], bf16, tag="ym", name=f"ym{xc}")
            nc.scalar.copy(ym[:], pm[:])
            # corner turn: 8 PE transposes, batched 4 per psum tile, then one
            # [128,512] strided eviction per tile spread across engines
            for j in range(2):
                pt = ppB.tile([128, 512], bf16, tag="pt", name=f"turn{xc}_{j}")
                for s4 in range(4):
                    s = j * 4 + s4
                    nc.tensor.transpose(
                        pt[:, s4 * 128:(s4 + 1) * 128],
                        ym[:, s * 128:(s + 1) * 128],
                        eyeb[:])
                # pt[tf, (s4,h,q,d)] -> a_t[tf, (g,d)], g = xc*32+h*16+q*8+s
                dst = a_t[:][:, (xc * 32) * DSH:(xc * 32 + 32) * DSH] \
                    .rearrange("p (hq blk d) -> p hq blk d", hq=4, blk=8)[
                        :, :, j * 4:(j + 1) * 4, :]
                psrc = pt[:].rearrange("p (s4 hq d) -> p hq s4 d", s4=4, hq=4)
                eng = _TURN_ENG[(2 * xc + j) % 2]
                if eng is nc.scalar:
                    eng.copy(dst, psrc)
                else:
                    eng.tensor_copy(dst, psrc)

        def precombine(b, wf_t, ztp, xwp, tmpp):
            """butterfly planes for frame batch b: zt cols (plane, qc, f, d);
            planes 0=y0, 1=y2, 2=m0, 3=m1.  Window applied via per-partition
            tensor_scalar on GpSimd; adds on DVE."""
            zt = ztp.tile([128, 4 * 4 * 512], bf16, tag="zt", name=f"zt{b}")
            for qc in range(4):
                xw = xwp.tile([128, 2048], bf16, tag="xw", name=f"xw{b}_{qc}")
                for j in range(4):
                    base = (128 * b + qc + 4 * j) * DSH
                    view = a_t[:][:, base: base + 4096] \
                        .rearrange("p (f q) -> p f q", f=16)[:, :, :DSH]
                    nc.gpsimd.tensor_scalar_mul(
                        xw[:, j * 512:(j + 1) * 512]
                        .rearrange("p (f d) -> p f d", f=16),
                        view, wcol[:, j * 4 + qc: j * 4 + qc + 1])
                tmp = tmpp.tile([128, 1024], bf16, tag="tmp", name=f"tm{b}_{qc}")
                nc.vector.tensor_add(tmp[:, :512], xw[:, 0:512], xw[:, 1024:1536])
                nc.vector.tensor_add(tmp[:, 512:], xw[:, 512:1024], xw[:, 1536:2048])
                z = lambda pl: zt[:, (pl * 4 + qc) * 512:(pl * 4 + qc + 1) * 512]
                nc.vector.tensor_sub(z(2), xw[:, 0:512], xw[:, 1024:1536])
                nc.vector.tensor_sub(z(3), xw[:, 512:1024], xw[:, 1536:2048])
                nc.vector.tensor_add(z(0), tmp[:, :512], tmp[:, 512:])
                nc.vector.tensor_sub(z(1), tmp[:, :512], tmp[:, 512:])
            return zt

        def fwd_batch(f16, wf_t, zt):
            # radix-4 forward: per m-block, accumulate plane x qc matmuls
            fc = f16
            blk = [0]
            for m in range(16):
                ps = sp.tile([128, 512], f32, tag="sm", name=f"sm{f16}_{m}")
                s = m // 4
                planes = _R4_PLANES[s]
                out_ap = ps[:].rearrange("p (d f) -> p f d", f=16)
                nmm = len(planes) * 4
                i = 0
                for pl in planes:
                    for qc in range(4):
                        rhs = zt[:, (pl * 4 + qc) * 512:(pl * 4 + qc + 1) * 512] \
                            .rearrange("p (f d) -> p f d", f=16)
                        nc.tensor.matmul(
                            out_ap,
                            wf_t[:, blk[0] * 128:(blk[0] + 1) * 128],
                            rhs,
                            start=(i == 0), stop=(i == nmm - 1))
                        blk[0] += 1
                        i += 1
                # per-m eviction (ScalarE; DVE is scanning)
                src_ = ps[:].rearrange("p (d f) -> p d f", f=16)
                doff = fc * SPECW + m * DSH * CB
                dst = spec[:][:, doff: doff + DSH * CB] \
                    .rearrange("p (d c) -> p d c", c=CB)[:, :, 1: 1 + FW]
                nc.scalar.copy(dst, src_)

        def scan_block(fc):
            # direct recurrence out[f] = (spec[f] + out[f-1]) * T via
            # state = (data0 + state) * data1, split DVE (m 0..8) / Pool
            # (m 9..15). pat col0 = 1 so col0 passes the injected carry.
            base = fc * SPECW
            nc.vector.tensor_tensor_scan(
                spec[:, base:base + SCAN_DVE],
                spec[:, base:base + SCAN_DVE],
                pat[:, 0:SCAN_DVE],
                0.0, Alu.add, Alu.mult)
            nc.vector.tensor_tensor_scan(
                spec[:, base + SCAN_DVE:base + SPECW],
                spec[:, base + SCAN_DVE:base + SPECW],
                pat[:, SCAN_DVE:SPECW],
                0.0, Alu.add, Alu.mult)

        def inject_block(fc):
            # pre-add the carry (out[15] of block fc) into block fc+1's
            # frame-0 column; must run AFTER fwd(fc+1)'s evictions have
            # written the raw col (they would clobber it otherwise) and
            # before scan(fc+1)
            base = fc * SPECW
            nmd_dve = SCAN_DVE // CB
            srcv = spec[:][:, base: base + SPECW] \
                .rearrange("p (md c) -> p md c", c=CB)[:, :, CB - 1: CB]
            dstv = spec[:][:, base + SPECW: base + 2 * SPECW] \
                .rearrange("p (md c) -> p md c", c=CB)[:, :, 1:2]
            nc.vector.tensor_add(dstv[:, :nmd_dve], dstv[:, :nmd_dve],
                                 srcv[:, :nmd_dve])
            nc.vector.tensor_add(dstv[:, nmd_dve:], dstv[:, nmd_dve:],
                                 srcv[:, nmd_dve:])

        # ================= phase F (+ scan), pipelined =================
        with ExitStack() as ctxF:
            wp = ctxF.enter_context(tc.tile_pool(name="wfp", bufs=1))
            # phase F PSUM: mixer pm [128,1024] x2 = 4 banks, fwd sp x3 = 3
            wf_t = wp.tile([128, 96 * 128], bf16, tag="wf")
            ztp = ctxF.enter_context(tc.tile_pool(name="ztp", bufs=2))
            xwp = ctxF.enter_context(tc.tile_pool(name="xwp", bufs=2))
            tmpp = ctxF.enter_context(tc.tile_pool(name="tmpp", bufs=2))
            sp = ctxF.enter_context(tc.tile_pool(name="sp", bufs=2, space="PSUM"))
            # mixer-side tiles freed after precombine(3) to make room for wi_t
            ctxA = ExitStack()
            pa = ctxA.enter_context(tc.tile_pool(name="pa", bufs=1))
            ppM = ctxA.enter_context(tc.tile_pool(name="ppM", bufs=2, space="PSUM"))
            ppB = ctxA.enter_context(tc.tile_pool(name="ppB", bufs=2, space="PSUM"))
            xin = ctxA.enter_context(tc.tile_pool(name="xin", bufs=3))
            ymp = ctxA.enter_context(tc.tile_pool(name="ymp", bufs=3))
            a_t = pa.tile([128, GPAD * DSH], bf16, tag="a")
            nc.vector.memset(a_t[:, GCH * DSH:], 0.0)

            nc.sync.dma_start(out=mx4[:], in_=mixwd[:])
            mixer_chunk(0)
            nc.sync.dma_start(out=eyeb[:], in_=eyebd[:])
            nc.sync.dma_start(out=wcol[:], in_=wcold[:])
            # weights on the Act queue (x streams on sync unimpeded), in
            # slices so x/weight transfers interleave on the DMA engines
            for s in range(8):
                nc.scalar.dma_start(out=wf_t[:, s * 1536:(s + 1) * 1536],
                                    in_=wf4[:, s * 1536:(s + 1) * 1536])
            nc.sync.dma_start(out=eye[:], in_=eyed[:])
            for xc in range(1, 5):
                mixer_chunk(xc)
            nc.scalar.dma_start(out=pat[:, :SPECW // 2], in_=patd[:, :SPECW // 2])
            nc.scalar.dma_start(out=pat[:, SPECW // 2:], in_=patd[:, SPECW // 2:])
            zt0 = precombine(0, wf_t, ztp, xwp, tmpp)
            for xc in range(5, 9):
                mixer_chunk(xc)
            fwd_batch(0, wf_t, zt0)
            zt1 = precombine(1, wf_t, ztp, xwp, tmpp)
            scan_block(0)
            for xc in range(9, 13):
                mixer_chunk(xc)
            fwd_batch(1, wf_t, zt1)
            inject_block(0)
            zt2 = precombine(2, wf_t, ztp, xwp, tmpp)
            scan_block(1)
            for xc in range(13, NXC):
                mixer_chunk(xc)
            fwd_batch(2, wf_t, zt2)
            inject_block(1)
            zt3 = precombine(3, wf_t, ztp, xwp, tmpp)
            ctxA.close()
            scan_block(2)
            fwd_batch(3, wf_t, zt3)
            inject_block(2)
            scan_block(3)

        # ================= phase I (radix-4 inverse) =================
        with ExitStack() as ctxI:
            wp2 = ctxI.enter_context(tc.tile_pool(name="wip2", bufs=1))
            # phase I PSUM: 6 plane psums + 2 emit-transpose psums = 8 banks
            zpl = ctxI.enter_context(tc.tile_pool(name="zpl", bufs=6, space="PSUM"))
            ppT = ctxI.enter_context(tc.tile_pool(name="ppT", bufs=2, space="PSUM"))
            wi_t = wp2.tile([128, 96 * 128], bf16, tag="wi")
            # sliced in consumption (qc) order: the first slice's ~2.2us
            # transfer is the only phase-I startup exposure
            for s in range(4):
                nc.scalar.dma_start(out=wi_t[:, s * 3072:(s + 1) * 3072],
                                    in_=wi4d[:, s * 3072:(s + 1) * 3072])
            wicol = wp2.tile([128, 16], f32, tag="wicol")
            nc.sync.dma_start(out=wicol[:], in_=wicold[:])
            ztail = wp2.tile([128, 2 * 4 * DSH], bf16, tag="ztail")
            nc.vector.memset(ztail[:], 0.0)

            tout = ctxI.enter_context(tc.tile_pool(name="tout", bufs=6))
            stg = ctxI.enter_context(tc.tile_pool(name="stg", bufs=3))
            zbp = ctxI.enter_context(tc.tile_pool(name="zbp", bufs=3))
            efp = ctxI.enter_context(tc.tile_pool(name="efp", bufs=3))
            aqp = ctxI.enter_context(tc.tile_pool(name="aqp", bufs=3))
            ywp = ctxI.enter_context(tc.tile_pool(name="ywp", bufs=3))
            ohp = ctxI.enter_context(tc.tile_pool(name="ohp", bufs=3))
            tailp = ctxI.enter_context(tc.tile_pool(name="tailp", bufs=2))

            yv = yout.rearrange("d (a4 fl t) -> fl d a4 t", fl=4, t=1024)

            st2_cur = {}

            def emit_store(tt, fc, qc, h):
                p4 = ppT.tile([128, 512], f32, tag="pp",
                              name=f"t4_{fc}_{qc}_{h}")
                for r2 in range(4):
                    nc.tensor.transpose(
                        p4[:, r2 * 128:(r2 + 1) * 128],
                        tt[:, r2 * 128:(r2 + 1) * 128],
                        eye[:])
                # stage 4 qc-groups into one [128, (r2, qc, t)] tile so the
                # y DMAs coarsen 4x (32 total instead of 128): the per-DMA
                # 650ns queue-issue cost was the phase-I drain bottleneck
                if (fc, h) not in st2_cur:
                    st2_cur[(fc, h)] = stg.tile([128, 2048], f32, tag="stg",
                                                name=f"st{fc}_{h}")
                st2 = st2_cur[(fc, h)]
                nc.scalar.copy(
                    st2[:].rearrange("p (r2 q t) -> p r2 q t",
                                     r2=4, q=4)[:, :, qc, :],
                    p4[:].rearrange("p (r2 t) -> p r2 t", r2=4))
                # all y DMAs on sync (head-parked DMAs must not block Act
                # compute); for the last fc, flush qc-pair halves so the
                # final drain overlaps the remaining compute
                if fc == FC - 1 and qc == 1:
                    for r2 in range(4):
                        dst = yv[:, :, 4 * fc + r2, 512 * h: 512 * h + 256]
                        nc.sync.dma_start(
                            out=dst,
                            in_=st2[:].rearrange(
                                "p (r2 q t) -> p r2 q t", r2=4, q=4)[
                                :, r2, 0:2, :])
                elif qc == 3:
                    if fc == FC - 1:
                        for r2 in range(4):
                            dst = yv[:, :, 4 * fc + r2,
                                     512 * h + 256: 512 * h + 512]
                            nc.sync.dma_start(
                                out=dst,
                                in_=st2[:].rearrange(
                                    "p (r2 q t) -> p r2 q t", r2=4, q=4)[
                                    :, r2, 2:4, :])
                    else:
                        for r2 in range(4):
                            dst = yv[:, :, 4 * fc + r2, 512 * h: 512 * h + 512]
                            nc.sync.dma_start(
                                out=dst, in_=st2[:, r2 * 512:(r2 + 1) * 512])
                    del st2_cur[(fc, h)]

            # Pool cannot read PSUM; fc0 avoids DVE (still draining scan(3))
            _ZB_ENG0 = [nc.scalar] * 6
            _ZB_ENG = [nc.scalar, nc.vector, nc.scalar,
                       nc.vector, nc.scalar, nc.scalar]
            deferred = []
            tail_prev = ztail
            for fc in range(FC):
                tail_new = tailp.tile([128, 2 * 4 * DSH], bf16, tag="tail",
                                      name=f"tail{fc}") if fc < FC - 1 else None
                for qc in range(4):
                    # 6 z-plane transforms: contraction over family slot-reals
                    zb = zbp.tile([128, 6 * 512], bf16, tag="zb",
                                  name=f"zb{fc}_{qc}")
                    for pl in range(6):
                        s = _INV_PLANES[pl][0]
                        ps = zpl.tile([128, 512], f32, tag="pp",
                                      name=f"zp{fc}_{qc}_{pl}")
                        out_ap = ps[:].rearrange("p (f d) -> p d f", f=FW)
                        for subm in range(4):
                            m = 4 * s + subm
                            base = fc * SPECW + m * DSH * CB
                            rhs = spec[:][:, base: base + DSH * CB] \
                                .rearrange("p (d c) -> p d c", c=CB)[:, :, 1: 1 + FW]
                            blk = (qc * 6 + pl) * 4 + subm
                            nc.tensor.matmul(
                                out_ap,
                                wi_t[:, blk * 128:(blk + 1) * 128],
                                rhs, start=(subm == 0), stop=(subm == 3))
                        eng = (_ZB_ENG0 if fc == 0 else _ZB_ENG)[pl]
                        if eng is nc.scalar:
                            eng.copy(zb[:, pl * 512:(pl + 1) * 512], ps[:])
                        else:
                            eng.tensor_copy(
                                zb[:, pl * 512:(pl + 1) * 512], ps[:])
                    # butterflies (DVE, bf16): e,f,gg,h then quarters a0..a3
                    ef = efp.tile([128, 4 * 512], bf16, tag="ef",
                                  name=f"ef{fc}_{qc}")
                    z = lambda pl: zb[:, pl * 512:(pl + 1) * 512]
                    nc.vector.tensor_add(ef[:, 0 * 512:1 * 512], z(0), z(3))   # e
                    nc.vector.tensor_sub(ef[:, 1 * 512:2 * 512], z(0), z(3))   # f
                    nc.vector.tensor_add(ef[:, 2 * 512:3 * 512], z(1), z(4))   # gg
                    nc.vector.tensor_sub(ef[:, 3 * 512:4 * 512], z(5), z(2))   # h
                    aq = aqp.tile([128, 4 * 512], bf16, tag="aq",
                                  name=f"aq{fc}_{qc}")
                    E, F_, G, H = (ef[:, i * 512:(i + 1) * 512] for i in range(4))
                    nc.vector.tensor_add(aq[:, 0 * 512:1 * 512], E, G)   # a0
                    nc.vector.tensor_add(aq[:, 1 * 512:2 * 512], F_, H)  # a1
                    nc.vector.tensor_sub(aq[:, 2 * 512:3 * 512], E, G)   # a2
                    nc.vector.tensor_sub(aq[:, 3 * 512:4 * 512], F_, H)  # a3
                    # save pre-window tail quarters (a2,a3 of frame 15)
                    if tail_new is not None:
                        for j2 in range(2):
                            nc.vector.tensor_copy(
                                tail_new[:, (j2 * 4 + qc) * DSH:
                                         (j2 * 4 + qc + 1) * DSH],
                                aq[:, (2 + j2) * 512 + 15 * DSH:
                                   (2 + j2) * 512 + 16 * DSH])
                    # window (GpSimd, per-partition scalars) + OLA + tanh
                    for h in range(2):
                        yw = ywp.tile([128, 1024], bf16, tag="yw",
                                      name=f"yw{fc}_{qc}_{h}")
                        nc.gpsimd.tensor_scalar_mul(
                            yw[:, :512], aq[:, h * 512:(h + 1) * 512],
                            wicol[:, h * 4 + qc: h * 4 + qc + 1])
                        nc.gpsimd.tensor_scalar_mul(
                            yw[:, 512:], aq[:, (h + 2) * 512:(h + 3) * 512],
                            wicol[:, (h + 2) * 4 + qc: (h + 2) * 4 + qc + 1])
                        # windowed tail quarter for frame 0 of this batch
                        wt = ywp.tile([128, DSH], bf16, tag="wt",
                                      name=f"wt{fc}_{qc}_{h}")
                        nc.gpsimd.tensor_scalar_mul(
                            wt[:], tail_prev[:, (h * 4 + qc) * DSH:
                                             (h * 4 + qc + 1) * DSH],
                            wicol[:, (h + 2) * 4 + qc: (h + 2) * 4 + qc + 1])
                        oh = ohp.tile([128, 512], bf16, tag="oh",
                                      name=f"oh{fc}_{qc}_{h}")
                        nc.vector.tensor_add(
                            oh[:, DSH:], yw[:, DSH:512], yw[:, 512:1024 - DSH])
                        nc.vector.tensor_add(oh[:, :DSH], yw[:, :DSH], wt[:])
                        # tanh now; corner-turn/store deferred 2 qc-groups
                        tt = tout.tile([128, 512], f32, tag="to",
                                       name=f"to{fc}_{qc}_{h}")
                        nc.scalar.activation(
                            tt[:], oh[:], mybir.ActivationFunctionType.Tanh)
                        deferred.append((tt, fc, qc, h))
                    keep = 4 if fc < FC - 1 else 0
                    while len(deferred) > keep:
                        emit_store(*deferred.pop(0))
                tail_prev = tail_new if tail_new is not None else ztail
            while deferred:
                emit_store(*deferred.pop(0))
    nc.compile()
    return nc


def _get_nc():
    global _CACHED_NC
    if _CACHED_NC is None:
        _CACHED_NC = _build_program()
    return _CACHED_NC


def kernel(x, transfer, mixer_matrix, gain, _trace=False):
    import ml_dtypes
    from concourse.bass_utils import run_bass_kernel_spmd

    x = np.ascontiguousarray(np.asarray(x, np.float32))
    transfer = np.asarray(transfer, np.float32)
    mixer_matrix = np.asarray(mixer_matrix, np.float32)
    gain = np.asarray(gain, np.float32)

    bf = ml_dtypes.bfloat16
    wf4_np = build_wf4().astype(bf)
    wcol_np = build_wcol().astype(np.float32)
    wi4_np = build_wi4().astype(bf)
    wicol_np = (float(gain.ravel()[0]) * build_wcol()).astype(np.float32)
    Ts = build_t_slots(transfer)
    eye = np.eye(128, dtype=np.float32)
    eyeb_np = np.eye(128, dtype=np.float64).astype(bf)

    in_maps = []
    for c in range(8):
        b, dh = c // 2, c % 2
        mixw4 = build_mixw4(
            np.asarray(mixer_matrix, np.float64)[:, dh * DSH:(dh + 1) * DSH]
        ).astype(bf)
        patc = build_pattern(Ts[dh * DSH:(dh + 1) * DSH]).astype(bf)
        in_maps.append({
            "xb": x[b].astype(bf),
            "mixw4": mixw4,
            "wf4": wf4_np,
            "wcol": wcol_np,
            "wi4": wi4_np,
            "wicol": wicol_np,
            "pat": patc,
            "eye": eye,
            "eyeb": eyeb_np,
        })

    nc = _get_nc()
    res = run_bass_kernel_spmd(nc, in_maps, list(range(8)), trace=_trace)
    out = np.zeros((BATCH, CPD, TIME), np.float32)
    for c in range(8):
        b, dh = c // 2, c % 2
        out[b, dh * DSH:(dh + 1) * DSH] = res.results[c]["y"]
    if _trace:
        return out, res
    return out



# revision 28
# speedup vs baseline: 1.0199x; 1.0199x over previous
"""Trainium2 Bass kernel for nn_Block_38517266710836.

reference pipeline: channel mixer -> STFT (hann 2048, hop 1024) -> per-frame
recurrence out[f] = (spec[f] + out[f-1]) * transfer -> iSTFT (hann synthesis)
-> overlap-add -> gain -> tanh.

Sharding: 8 cores, data-parallel over (batch, channel-half): core c handles
batch c//2, mixed channels [32*(c%2), +32). Each core receives its batch's
full 64-channel input (the mixer contracts channels) and writes 32 rows.

Pipelined single-pass program per core: mixer chunks, forward-DFT frame
batches, the DVE recurrence scan, and the inverse/overlap-add phase are
interleaved so the scan and evictions hide under PE matmul work.  Forward
evictions and corner-turn copies run on ScalarE (DVE is reserved for the
scan), weights stream on the gpsimd DMA queue, x/y on the sync queue, and
PSUM pools are shared across phases to fit the 8-bank budget.
"""

import numpy as np

WINDOW = 2048
STEP = 1024
CPD = 64
BATCH = 4
TIME = 65536
FRAMES = 64
NJ = 16              # per-frame time chunks (fwd contraction blocks)
NM = 16              # spectral slot chunks
DSH = 32             # mixed channels per core
GCH = TIME // 128    # 512 global 128-sample chunks
GPAD = GCH + 16      # + zero pad (frame 63 reaches t=66560; extra width so
                     # the forward rhs slice [base, base+2048) stays in-bounds)
FC = 4               # frame chunks for the scan layout
FW = 16              # frames per chunk
CB = 17              # chain block: 1 inject/reset col + 16 frame cols
SPECW = NM * DSH * CB  # 8704 free cols per fc block


def _hann(n):
    return (0.5 - 0.5 * np.cos(2.0 * np.pi * np.arange(n) / n)).astype(np.float64)


def _slot_tables():
    """slot s in [0,2048): s<1024 -> Re[k=s]; s==1024 -> Re[1024] (parked in
    Im[0]'s slot, since Im[0] is identically 0); s>1024 -> Im[k=s-1024]."""
    k_of_slot = np.zeros(2048, np.int64)
    is_im = np.zeros(2048, np.bool_)
    for s in range(2048):
        if s < 1024:
            k_of_slot[s] = s
        elif s == 1024:
            k_of_slot[s] = 1024
        else:
            k_of_slot[s] = s - 1024
            is_im[s] = True
    return k_of_slot, is_im


def build_fwd_weights():
    """[2048 n, 2048 slots]: windowed rfft of one frame, slot layout."""
    n = np.arange(WINDOW, dtype=np.float64)
    w = _hann(WINDOW)
    k_of_slot, is_im = _slot_tables()
    ang = 2.0 * np.pi * np.outer(n, k_of_slot.astype(np.float64)) / WINDOW
    W = np.where(is_im[None, :], -np.sin(ang), np.cos(ang))
    W *= w[:, None]
    return W


def build_inv_weights(gain):
    """[2048 slots, 2048 n]: gain * hann * irfft from slot layout."""
    n = np.arange(WINDOW, dtype=np.float64)
    w = _hann(WINDOW)
    k_of_slot, is_im = _slot_tables()
    ang = 2.0 * np.pi * np.outer(k_of_slot.astype(np.float64), n) / WINDOW
    k = k_of_slot
    re_coef = (2.0 - (k == 0) - (k == 1024))[:, None] / WINDOW * np.cos(ang)
    im_coef = -2.0 / WINDOW * np.sin(ang)
    W = np.where(is_im[:, None], im_coef, re_coef)
    W[1024, :] = np.cos(np.pi * n) / WINDOW
    W *= (gain * w)[None, :]
    return W


def _slot_tables_r4():
    """family-major slot layout: slot' = s*512 + local; family s holds
    k = s, s+4, ... <= 1024(ish), (re, im) interleaved k-major."""
    karr = np.zeros(2048, np.int64)
    isim = np.zeros(2048, np.bool_)
    pos = 0
    for s in range(4):
        for k in range(s, 1025, 4):
            karr[pos] = k; isim[pos] = False; pos += 1
            if k not in (0, 1024):
                karr[pos] = k; isim[pos] = True; pos += 1
    assert pos == 2048
    return karr, isim


_R4_PLANES = {0: [0], 1: [2, 3], 2: [1], 3: [2, 3]}  # m//4 -> plane list


def _build_wfam():
    """family -> list of (plane, [512 q, 512 r]) weight matrices.
    planes: 0=y0, 1=y2, 2=m0, 3=m1."""
    q = np.arange(512, dtype=np.float64)
    karr, isim = _slot_tables_r4()
    fams = {}
    for s in range(4):
        kv = karr[s * 512:(s + 1) * 512].astype(np.float64)
        iv = isim[s * 512:(s + 1) * 512]
        ang = 2.0 * np.pi * np.outer(q, kv) / WINDOW
        c, sn = np.cos(ang), np.sin(ang)
        if s == 0:
            fams[s] = [(0, np.where(iv[None, :], -sn, c))]
        elif s == 2:
            fams[s] = [(1, np.where(iv[None, :], -sn, c))]
        elif s == 1:
            fams[s] = [(2, np.where(iv[None, :], -sn, c)),
                       (3, np.where(iv[None, :], -c, -sn))]
        else:
            fams[s] = [(2, np.where(iv[None, :], -sn, c)),
                       (3, np.where(iv[None, :], c, sn))]
    return fams


def build_wf4():
    """[128, 96*128] SBUF-ready block layout matching the device MM loop:
    for qp, mi: m=2qp+mi -> (s=m//4, subm=m%4): for plane, for qc: block
    = Wfam[s][plane][qc*128:+128, subm*128:+128]."""
    fams = _build_wfam()
    blocks = []
    for qp in range(8):
        for mi in range(2):
            m = qp * 2 + mi
            s, subm = m // 4, m % 4
            for pl, Wm in fams[s]:
                for qc in range(4):
                    blocks.append(Wm[qc * 128:(qc + 1) * 128,
                                     subm * 128:(subm + 1) * 128])
    return np.concatenate(blocks, axis=1)  # [128, 96*128]


def build_wcol():
    """[128, 16] per-partition window scalars: col j*4+qc = w[qc*128+p+512j]."""
    w = _hann(WINDOW)
    out = np.zeros((128, 16), np.float64)
    for j in range(4):
        for qc in range(4):
            out[:, j * 4 + qc] = w[qc * 128 + np.arange(128) + 512 * j]
    return out


def build_t_slots(transfer):
    karr, _ = _slot_tables_r4()
    return np.asarray(transfer, np.float64)[:, karr]  # [ch, 2048]


_INV_PLANES = [(0, False), (1, False), (1, True), (2, False), (3, False), (3, True)]
# zb plane order: 0=zre0, 1=zre1, 2=zim1, 3=zre2, 4=zre3, 5=zim3


def build_wi4():
    """[128, 96*128] inverse z-plane weights; device order:
    for qc in 4: for pl in 6: for subm in 4."""
    karr, isim = _slot_tables_r4()
    q = np.arange(512, dtype=np.float64)
    Vs = []
    for (s, want_im) in _INV_PLANES:
        kv = karr[s * 512:(s + 1) * 512].astype(np.float64)
        iv = isim[s * 512:(s + 1) * 512]
        coef = (2.0 - (kv == 0) - (kv == 1024)) / WINDOW
        ang = 2.0 * np.pi * np.outer(kv, q) / WINDOW
        c, sn = np.cos(ang), np.sin(ang)
        V = coef[:, None] * (np.where(iv[:, None], c, sn) if want_im
                             else np.where(iv[:, None], -sn, c))
        Vs.append(V)  # [512 slot-reals, 512 q]
    blocks = []
    for qc in range(4):
        for V in Vs:
            for subm in range(4):
                blocks.append(V[subm * 128:(subm + 1) * 128,
                               qc * 128:(qc + 1) * 128])
    return np.concatenate(blocks, axis=1)


def build_inv_weights_perm(gain):
    """[2048 slots', 2048 n]: gain * hann * irfft from the r4 slot layout."""
    n = np.arange(WINDOW, dtype=np.float64)
    w = _hann(WINDOW)
    karr, isim = _slot_tables_r4()
    k = karr.astype(np.float64)
    ang = 2.0 * np.pi * np.outer(k, n) / WINDOW
    re_coef = (2.0 - (karr == 0) - (karr == 1024))[:, None] / WINDOW * np.cos(ang)
    im_coef = -2.0 / WINDOW * np.sin(ang)
    W = np.where(isim[:, None], im_coef, re_coef)
    W *= (gain * w)[None, :]
    return W


def build_fwd_weights_perm():
    """effective [2048 n, 2048 slots'] fwd matrix (validation only)."""
    n = np.arange(WINDOW, dtype=np.float64)
    w = _hann(WINDOW)
    karr, isim = _slot_tables_r4()
    ang = 2.0 * np.pi * np.outer(n, karr.astype(np.float64)) / WINDOW
    W = np.where(isim[None, :], -np.sin(ang), np.cos(ang))
    W *= w[:, None]
    return W


def build_pattern(t_slots_core):
    """T-pattern [128, SPECW]: per (m,d) chain block of CB cols:
    col 0 = 0 (reset: state=(x+state)*0), cols 1..16 = T[slot(m,kf), d]
    for the scan state=(spec+state)*pat; the carry between fc blocks is
    pre-added into the next block's frame-0 column."""
    pat = np.zeros((128, SPECW), np.float64)
    for m in range(NM):
        for d in range(DSH):
            base = (m * DSH + d) * CB
            pat[:, base + 1: base + CB] = \
                t_slots_core[d, m * 128:(m + 1) * 128][:, None]
    return pat


def build_mixw4(mix_half):
    """[128, 64] block-diag lhsT: rows (q,c) q in 2, cols (q',d):
    M[c, d] if q==q' else 0."""
    out = np.zeros((128, 64), np.float64)
    out[0:64, 0:32] = mix_half
    out[64:128, 32:64] = mix_half
    return out


def emulate(x, transfer, mixer_matrix, gain, wdtype=np.float32):
    """Numpy emulation of the device math (offline validation)."""
    b, c, t = x.shape
    Wf = build_fwd_weights_perm().astype(wdtype).astype(np.float64)
    Wi = build_inv_weights_perm(float(np.asarray(gain).ravel()[0])).astype(wdtype).astype(np.float64)
    Ts = build_t_slots(transfer)
    y = np.einsum('bct,cd->bdt', np.asarray(x, np.float64),
                  np.asarray(mixer_matrix, np.float64))
    yp = np.pad(y, ((0, 0), (0, 0), (0, STEP)))
    out = np.zeros((b, c, t), np.float64)
    for bi in range(b):
        frames = np.stack([yp[bi, :, f * STEP: f * STEP + WINDOW]
                           for f in range(FRAMES)], 1)
        spec = frames.astype(wdtype).astype(np.float64) @ Wf
        st = np.zeros((c, 2048))
        outs = np.zeros_like(spec)
        for f in range(FRAMES):
            st = (spec[:, f].astype(wdtype).astype(np.float64) + st) * Ts
            outs[:, f] = st
        aud = outs.astype(wdtype).astype(np.float64) @ Wi
        acc = np.zeros((c, t + STEP))
        for f in range(FRAMES):
            acc[:, f * STEP: f * STEP + WINDOW] += aud[:, f]
        out[bi] = np.tanh(acc[:, :t])
    return out.astype(np.float32)


# ---------------------------------------------------------------------------
# Device program
# ---------------------------------------------------------------------------

_CACHED_NC = None


def _build_program():
    import concourse.bacc as bacc
    import concourse.mybir as mybir
    from concourse import tile
    from contextlib import ExitStack

    f32 = mybir.dt.float32
    bf16 = mybir.dt.bfloat16
    Alu = mybir.AluOpType

    nc = bacc.Bacc("TRN2", target_bir_lowering=False, debug=False, num_devices=8)
    xb = nc.dram_tensor("xb", [CPD, TIME], bf16, kind="ExternalInput").ap()
    mixwd = nc.dram_tensor("mixw4", [128, 64], bf16, kind="ExternalInput").ap()
    eyebd = nc.dram_tensor("eyeb", [128, 128], bf16, kind="ExternalInput").ap()
    wf4 = nc.dram_tensor("wf4", [128, 96 * 128], bf16, kind="ExternalInput").ap()
    wcold = nc.dram_tensor("wcol", [128, 16], f32, kind="ExternalInput").ap()
    wi4d = nc.dram_tensor("wi4", [128, 96 * 128], bf16, kind="ExternalInput").ap()
    wicold = nc.dram_tensor("wicol", [128, 16], f32, kind="ExternalInput").ap()
    patd = nc.dram_tensor("pat", [128, SPECW], bf16, kind="ExternalInput").ap()
    eyed = nc.dram_tensor("eye", [128, 128], f32, kind="ExternalInput").ap()
    yout = nc.dram_tensor("y", [DSH, TIME], f32, kind="ExternalOutput").ap()

    XCH = 4096           # x streamed in 4096-sample chunks
    NXC = TIME // XCH    # 16
    # scan split: DVE takes m-blocks 0..8, Pool 9..15
    SCAN_DVE = 9 * DSH * CB   # 4896 cols
    # x view [nx, q, c, h, f]: t = nx*4096 + h*2048 + q*1024 + f
    xbv = xb.rearrange("c (nx h q f) -> nx q c h f", nx=NXC, h=2, q=2)

    with tile.TileContext(nc) as tc, ExitStack() as ctx:
        persist = ctx.enter_context(tc.tile_pool(name="persist", bufs=1))
        spec = persist.tile([128, FC * SPECW], bf16, tag="spec")
        pat = persist.tile([128, SPECW], bf16, tag="pat")
        mx4 = persist.tile([128, 64], bf16, tag="mx4")
        eyeb = persist.tile([128, 128], bf16, tag="eyeb")
        eye = persist.tile([128, 128], f32, tag="eye")
        wcol = persist.tile([128, 16], f32, tag="wcol")

        xin = ctx.enter_context(tc.tile_pool(name="xin", bufs=3))
        ymp = ctx.enter_context(tc.tile_pool(name="ymp", bufs=3))



        # reset cols (col 0 of every chain) must not contain NaN garbage:
        # (garbage + state) * 0 is 0 only for finite garbage
        nc.vector.memset(
            spec[:].rearrange("p (fcmd c) -> p fcmd c", c=CB)[:, :, 0:1], 0.0)

        # Pool/gpsimd cannot access PSUM (walrus birverifier) — psum
        # evictions go on Act/DVE only
        _TURN_ENG = [nc.scalar, nc.vector]

        def mixer_chunk(xc):
            # xt partitions (q,c), free (h,f): t = xc*4096 + h*2048 + q*1024 + f
            xt = xin.tile([128, 2048], bf16, tag="x", name=f"x{xc}")
            for q in range(2):
                nc.sync.dma_start(
                    out=xt[q * 64:(q + 1) * 64, :].rearrange(
                        "c (h f) -> c h f", h=2),
                    in_=xbv[xc, q])
            pm = ppM.tile([128, 1024], f32, tag="pp", name=f"mix{xc}")
            for h in range(2):
                for hf in range(2):  # psum-bank-sized output halves
                    nc.tensor.matmul(
                        pm[h * 64:(h + 1) * 64,
                           hf * 512:(hf + 1) * 512],
                        mx4[:],
                        xt[:, h * 1024 + hf * 512: h * 1024 + (hf + 1) * 512],
                        start=True, stop=True)
            # ym[(h,q,d), f]
            ym = ymp.tile([128, 1024], bf16, tag="ym", name=f"ym{xc}")
            nc.scalar.copy(ym[:], pm[:])
            # corner turn: 8 PE transposes, batched 4 per psum tile, then one
            # [128,512] strided eviction per tile spread across engines
            for j in range(2):
                pt = ppB.tile([128, 512], bf16, tag="pt", name=f"turn{xc}_{j}")
                for s4 in range(4):
                    s = j * 4 + s4
                    nc.tensor.transpose(
                        pt[:, s4 * 128:(s4 + 1) * 128],
                        ym[:, s * 128:(s + 1) * 128],
                        eyeb[:])
                # pt[tf, (s4,h,q,d)] -> a_t[tf, (g,d)], g = xc*32+h*16+q*8+s
                dst = a_t[:][:, (xc * 32) * DSH:(xc * 32 + 32) * DSH] \
                    .rearrange("p (hq blk d) -> p hq blk d", hq=4, blk=8)[
                        :, :, j * 4:(j + 1) * 4, :]
                psrc = pt[:].rearrange("p (s4 hq d) -> p hq s4 d", s4=4, hq=4)
                eng = _TURN_ENG[(2 * xc + j) % 2]
                if eng is nc.scalar:
                    eng.copy(dst, psrc)
                else:
                    eng.tensor_copy(dst, psrc)

        def precombine(b, wf_t, ztp, xwp, tmpp):
            """butterfly planes for frame batch b: zt cols (plane, qc, f, d);
            planes 0=y0, 1=y2, 2=m0, 3=m1.  Window applied via per-partition
            tensor_scalar on GpSimd; adds on DVE."""
            zt = ztp.tile([128, 4 * 4 * 512], bf16, tag="zt", name=f"zt{b}")
            for qc in range(4):
                xw = xwp.tile([128, 2048], bf16, tag="xw", name=f"xw{b}_{qc}")
                for j in range(4):
                    base = (128 * b + qc + 4 * j) * DSH
                    view = a_t[:][:, base: base + 4096] \
                        .rearrange("p (f q) -> p f q", f=16)[:, :, :DSH]
                    nc.gpsimd.tensor_scalar_mul(
                        xw[:, j * 512:(j + 1) * 512]
                        .rearrange("p (f d) -> p f d", f=16),
                        view, wcol[:, j * 4 + qc: j * 4 + qc + 1])
                tmp = tmpp.tile([128, 1024], bf16, tag="tmp", name=f"tm{b}_{qc}")
                nc.vector.tensor_add(tmp[:, :512], xw[:, 0:512], xw[:, 1024:1536])
                nc.vector.tensor_add(tmp[:, 512:], xw[:, 512:1024], xw[:, 1536:2048])
                z = lambda pl: zt[:, (pl * 4 + qc) * 512:(pl * 4 + qc + 1) * 512]
                nc.vector.tensor_sub(z(2), xw[:, 0:512], xw[:, 1024:1536])
                nc.vector.tensor_sub(z(3), xw[:, 512:1024], xw[:, 1536:2048])
                nc.vector.tensor_add(z(0), tmp[:, :512], tmp[:, 512:])
                nc.vector.tensor_sub(z(1), tmp[:, :512], tmp[:, 512:])
            return zt

        def fwd_batch(f16, wf_t, zt):
            # radix-4 forward: per m-block, accumulate plane x qc matmuls
            fc = f16
            blk = [0]
            for m in range(16):
                ps = sp.tile([128, 512], f32, tag="sm", name=f"sm{f16}_{m}")
                s = m // 4
                planes = _R4_PLANES[s]
                out_ap = ps[:].rearrange("p (d f) -> p f d", f=16)
                nmm = len(planes) * 4
                i = 0
                for pl in planes:
                    for qc in range(4):
                        rhs = zt[:, (pl * 4 + qc) * 512:(pl * 4 + qc + 1) * 512] \
                            .rearrange("p (f d) -> p f d", f=16)
                        nc.tensor.matmul(
                            out_ap,
                            wf_t[:, blk[0] * 128:(blk[0] + 1) * 128],
                            rhs,
                            start=(i == 0), stop=(i == nmm - 1))
                        blk[0] += 1
                        i += 1
                # per-m eviction (ScalarE; DVE is scanning)
                src_ = ps[:].rearrange("p (d f) -> p d f", f=16)
                doff = fc * SPECW + m * DSH * CB
                dst = spec[:][:, doff: doff + DSH * CB] \
                    .rearrange("p (d c) -> p d c", c=CB)[:, :, 1: 1 + FW]
                nc.scalar.copy(dst, src_)

        def scan_block(fc):
            # direct recurrence out[f] = (spec[f] + out[f-1]) * T via
            # state = (data0 + state) * data1, split DVE (m 0..8) / Pool
            # (m 9..15). pat col0 = 1 so col0 passes the injected carry.
            base = fc * SPECW
            nc.vector.tensor_tensor_scan(
                spec[:, base:base + SCAN_DVE],
                spec[:, base:base + SCAN_DVE],
                pat[:, 0:SCAN_DVE],
                0.0, Alu.add, Alu.mult)
            nc.vector.tensor_tensor_scan(
                spec[:, base + SCAN_DVE:base + SPECW],
                spec[:, base + SCAN_DVE:base + SPECW],
                pat[:, SCAN_DVE:SPECW],
                0.0, Alu.add, Alu.mult)

        def inject_block(fc):
            # pre-add the carry (out[15] of block fc) into block fc+1's
            # frame-0 column; must run AFTER fwd(fc+1)'s evictions have
            # written the raw col (they would clobber it otherwise) and
            # before scan(fc+1)
            base = fc * SPECW
            nmd_dve = SCAN_DVE // CB
            srcv = spec[:][:, base: base + SPECW] \
                .rearrange("p (md c) -> p md c", c=CB)[:, :, CB - 1: CB]
            dstv = spec[:][:, base + SPECW: base + 2 * SPECW] \
                .rearrange("p (md c) -> p md c", c=CB)[:, :, 1:2]
            nc.vector.tensor_add(dstv[:, :nmd_dve], dstv[:, :nmd_dve],
                                 srcv[:, :nmd_dve])
            nc.vector.tensor_add(dstv[:, nmd_dve:], dstv[:, nmd_dve:],
                                 srcv[:, nmd_dve:])

        # ================= phase F (+ scan), pipelined =================
        with ExitStack() as ctxF:
            wp = ctxF.enter_context(tc.tile_pool(name="wfp", bufs=1))
            # phase F PSUM: mixer pm [128,1024] x2 = 4 banks, fwd sp x3 = 3
            wf_t = wp.tile([128, 96 * 128], bf16, tag="wf")
            ztp = ctxF.enter_context(tc.tile_pool(name="ztp", bufs=2))
            xwp = ctxF.enter_context(tc.tile_pool(name="xwp", bufs=2))
            tmpp = ctxF.enter_context(tc.tile_pool(name="tmpp", bufs=2))
            sp = ctxF.enter_context(tc.tile_pool(name="sp", bufs=2, space="PSUM"))
            # mixer-side tiles freed after precombine(3) to make room for wi_t
            ctxA = ExitStack()
            pa = ctxA.enter_context(tc.tile_pool(name="pa", bufs=1))
            ppM = ctxA.enter_context(tc.tile_pool(name="ppM", bufs=2, space="PSUM"))
            ppB = ctxA.enter_context(tc.tile_pool(name="ppB", bufs=2, space="PSUM"))
            xin = ctxA.enter_context(tc.tile_pool(name="xin", bufs=3))
            ymp = ctxA.enter_context(tc.tile_pool(name="ymp", bufs=3))
            a_t = pa.tile([128, GPAD * DSH], bf16, tag="a")
            nc.vector.memset(a_t[:, GCH * DSH:], 0.0)

            nc.sync.dma_start(out=mx4[:], in_=mixwd[:])
            mixer_chunk(0)
            nc.sync.dma_start(out=eyeb[:], in_=eyebd[:])
            nc.sync.dma_start(out=wcol[:], in_=wcold[:])
            # weights on the Act queue (x streams on sync unimpeded), in
            # slices so x/weight transfers interleave on the DMA engines
            for s in range(8):
                nc.scalar.dma_start(out=wf_t[:, s * 1536:(s + 1) * 1536],
                                    in_=wf4[:, s * 1536:(s + 1) * 1536])
            nc.sync.dma_start(out=eye[:], in_=eyed[:])
            for xc in range(1, 5):
                mixer_chunk(xc)
            nc.scalar.dma_start(out=pat[:, :SPECW // 2], in_=patd[:, :SPECW // 2])
            nc.scalar.dma_start(out=pat[:, SPECW // 2:], in_=patd[:, SPECW // 2:])
            zt0 = precombine(0, wf_t, ztp, xwp, tmpp)
            for xc in range(5, 9):
                mixer_chunk(xc)
            fwd_batch(0, wf_t, zt0)
            zt1 = precombine(1, wf_t, ztp, xwp, tmpp)
            scan_block(0)
            for xc in range(9, 13):
                mixer_chunk(xc)
            fwd_batch(1, wf_t, zt1)
            inject_block(0)
            zt2 = precombine(2, wf_t, ztp, xwp, tmpp)
            scan_block(1)
            for xc in range(13, NXC):
                mixer_chunk(xc)
            fwd_batch(2, wf_t, zt2)
            inject_block(1)
            zt3 = precombine(3, wf_t, ztp, xwp, tmpp)
            ctxA.close()
            scan_block(2)
            fwd_batch(3, wf_t, zt3)
            inject_block(2)
            scan_block(3)

        # ================= phase I (radix-4 inverse) =================
        with ExitStack() as ctxI:
            wp2 = ctxI.enter_context(tc.tile_pool(name="wip2", bufs=1))
            # phase I PSUM: 6 plane psums + 2 emit-transpose psums = 8 banks
            zpl = ctxI.enter_context(tc.tile_pool(name="zpl", bufs=6, space="PSUM"))
            ppT = ctxI.enter_context(tc.tile_pool(name="ppT", bufs=2, space="PSUM"))
            wi_t = wp2.tile([128, 96 * 128], bf16, tag="wi")
            # sliced in consumption (qc) order: the first slice's ~2.2us
            # transfer is the only phase-I startup exposure
            for s in range(4):
                nc.scalar.dma_start(out=wi_t[:, s * 3072:(s + 1) * 3072],
                                    in_=wi4d[:, s * 3072:(s + 1) * 3072])
            wicol = wp2.tile([128, 16], f32, tag="wicol")
            nc.sync.dma_start(out=wicol[:], in_=wicold[:])
            ztail = wp2.tile([128, 2 * 4 * DSH], bf16, tag="ztail")
            nc.vector.memset(ztail[:], 0.0)

            tout = ctxI.enter_context(tc.tile_pool(name="tout", bufs=6))
            stg = ctxI.enter_context(tc.tile_pool(name="stg", bufs=3))
            zbp = ctxI.enter_context(tc.tile_pool(name="zbp", bufs=3))
            efp = ctxI.enter_context(tc.tile_pool(name="efp", bufs=3))
            aqp = ctxI.enter_context(tc.tile_pool(name="aqp", bufs=3))
            ywp = ctxI.enter_context(tc.tile_pool(name="ywp", bufs=3))
            ohp = ctxI.enter_context(tc.tile_pool(name="ohp", bufs=3))
            tailp = ctxI.enter_context(tc.tile_pool(name="tailp", bufs=2))

            yv = yout.rearrange("d (a4 fl t) -> fl d a4 t", fl=4, t=1024)

            st2_cur = {}

            def emit_store(tt, fc, qc, h):
                p4 = ppT.tile([128, 512], f32, tag="pp",
                              name=f"t4_{fc}_{qc}_{h}")
                for r2 in range(4):
                    nc.tensor.transpose(
                        p4[:, r2 * 128:(r2 + 1) * 128],
                        tt[:, r2 * 128:(r2 + 1) * 128],
                        eye[:])
                # stage 4 qc-groups into one [128, (r2, qc, t)] tile so the
                # y DMAs coarsen 4x (32 total instead of 128): the per-DMA
                # 650ns queue-issue cost was the phase-I drain bottleneck
                if (fc, h) not in st2_cur:
                    st2_cur[(fc, h)] = stg.tile([128, 2048], f32, tag="stg",
                                                name=f"st{fc}_{h}")
                st2 = st2_cur[(fc, h)]
                nc.scalar.copy(
                    st2[:].rearrange("p (r2 q t) -> p r2 q t",
                                     r2=4, q=4)[:, :, qc, :],
                    p4[:].rearrange("p (r2 t) -> p r2 t", r2=4))
                # all y DMAs on sync (head-parked DMAs must not block Act
                # compute); for the last fc, flush qc-pair halves so the
                # final drain overlaps the remaining compute
                if fc == FC - 1 and qc == 1:
                    for r2 in range(4):
                        dst = yv[:, :, 4 * fc + r2, 512 * h: 512 * h + 256]
                        nc.sync.dma_start(
                            out=dst,
                            in_=st2[:].rearrange(
                                "p (r2 q t) -> p r2 q t", r2=4, q=4)[
                                :, r2, 0:2, :])
                elif qc == 3:
                    if fc == FC - 1:
                        for r2 in range(4):
                            dst = yv[:, :, 4 * fc + r2,
                                     512 * h + 256: 512 * h + 512]
                            nc.sync.dma_start(
                                out=dst,
                                in_=st2[:].rearrange(
                                    "p (r2 q t) -> p r2 q t", r2=4, q=4)[
                                    :, r2, 2:4, :])
                    else:
                        for r2 in range(4):
                            dst = yv[:, :, 4 * fc + r2, 512 * h: 512 * h + 512]
                            eng = nc.sync if r2 < 2 else nc.scalar
                            eng.dma_start(
                                out=dst, in_=st2[:, r2 * 512:(r2 + 1) * 512])
                    del st2_cur[(fc, h)]

            # Pool cannot read PSUM; fc0 avoids DVE (still draining scan(3))
            _ZB_ENG0 = [nc.scalar] * 6
            _ZB_ENG = [nc.scalar, nc.vector, nc.scalar,
                       nc.vector, nc.scalar, nc.scalar]
            deferred = []
            tail_prev = ztail
            for fc in range(FC):
                tail_new = tailp.tile([128, 2 * 4 * DSH], bf16, tag="tail",
                                      name=f"tail{fc}") if fc < FC - 1 else None
                for qc in range(4):
                    # 6 z-plane transforms: contraction over family slot-reals
                    zb = zbp.tile([128, 6 * 512], bf16, tag="zb",
                                  name=f"zb{fc}_{qc}")
                    for pl in range(6):
                        s = _INV_PLANES[pl][0]
                        ps = zpl.tile([128, 512], f32, tag="pp",
                                      name=f"zp{fc}_{qc}_{pl}")
                        out_ap = ps[:].rearrange("p (f d) -> p d f", f=FW)
                        for subm in range(4):
                            m = 4 * s + subm
                            base = fc * SPECW + m * DSH * CB
                            rhs = spec[:][:, base: base + DSH * CB] \
                                .rearrange("p (d c) -> p d c", c=CB)[:, :, 1: 1 + FW]
                            blk = (qc * 6 + pl) * 4 + subm
                            nc.tensor.matmul(
                                out_ap,
                                wi_t[:, blk * 128:(blk + 1) * 128],
                                rhs, start=(subm == 0), stop=(subm == 3))
                        eng = (_ZB_ENG0 if fc == 0 else _ZB_ENG)[pl]
                        if eng is nc.scalar:
                            eng.copy(zb[:, pl * 512:(pl + 1) * 512], ps[:])
                        else:
                            eng.tensor_copy(
                                zb[:, pl * 512:(pl + 1) * 512], ps[:])
                    # butterflies (DVE, bf16): e,f,gg,h then quarters a0..a3
                    ef = efp.tile([128, 4 * 512], bf16, tag="ef",
                                  name=f"ef{fc}_{qc}")
                    z = lambda pl: zb[:, pl * 512:(pl + 1) * 512]
                    nc.vector.tensor_add(ef[:, 0 * 512:1 * 512], z(0), z(3))   # e
                    nc.vector.tensor_sub(ef[:, 1 * 512:2 * 512], z(0), z(3))   # f
                    nc.vector.tensor_add(ef[:, 2 * 512:3 * 512], z(1), z(4))   # gg
                    nc.vector.tensor_sub(ef[:, 3 * 512:4 * 512], z(5), z(2))   # h
                    aq = aqp.tile([128, 4 * 512], bf16, tag="aq",
                                  name=f"aq{fc}_{qc}")
                    E, F_, G, H = (ef[:, i * 512:(i + 1) * 512] for i in range(4))
                    nc.vector.tensor_add(aq[:, 0 * 512:1 * 512], E, G)   # a0
                    nc.vector.tensor_add(aq[:, 1 * 512:2 * 512], F_, H)  # a1
                    nc.vector.tensor_sub(aq[:, 2 * 512:3 * 512], E, G)   # a2
                    nc.vector.tensor_sub(aq[:, 3 * 512:4 * 512], F_, H)  # a3
                    # save pre-window tail quarters (a2,a3 of frame 15)
                    if tail_new is not None:
                        for j2 in range(2):
                            nc.vector.tensor_copy(
                                tail_new[:, (j2 * 4 + qc) * DSH:
                                         (j2 * 4 + qc + 1) * DSH],
                                aq[:, (2 + j2) * 512 + 15 * DSH:
                                   (2 + j2) * 512 + 16 * DSH])
                    # window (GpSimd, per-partition scalars) + OLA + tanh
                    for h in range(2):
                        yw = ywp.tile([128, 1024], bf16, tag="yw",
                                      name=f"yw{fc}_{qc}_{h}")
                        nc.gpsimd.tensor_scalar_mul(
                            yw[:, :512], aq[:, h * 512:(h + 1) * 512],
                            wicol[:, h * 4 + qc: h * 4 + qc + 1])
                        nc.gpsimd.tensor_scalar_mul(
                            yw[:, 512:], aq[:, (h + 2) * 512:(h + 3) * 512],
                            wicol[:, (h + 2) * 4 + qc: (h + 2) * 4 + qc + 1])
                        # windowed tail quarter for frame 0 of this batch
                        wt = ywp.tile([128, DSH], bf16, tag="wt",
                                      name=f"wt{fc}_{qc}_{h}")
                        nc.gpsimd.tensor_scalar_mul(
                            wt[:], tail_prev[:, (h * 4 + qc) * DSH:
                                             (h * 4 + qc + 1) * DSH],
                            wicol[:, (h + 2) * 4 + qc: (h + 2) * 4 + qc + 1])
                        oh = ohp.tile([128, 512], bf16, tag="oh",
                                      name=f"oh{fc}_{qc}_{h}")
                        nc.vector.tensor_add(
                            oh[:, DSH:], yw[:, DSH:512], yw[:, 512:1024 - DSH])
                        nc.vector.tensor_add(oh[:, :DSH], yw[:, :DSH], wt[:])
                        # tanh now; corner-turn/store deferred 2 qc-groups
                        tt = tout.tile([128, 512], f32, tag="to",
                                       name=f"to{fc}_{qc}_{h}")
                        nc.scalar.activation(
                            tt[:], oh[:], mybir.ActivationFunctionType.Tanh)
                        deferred.append((tt, fc, qc, h))
                    keep = 4 if fc < FC - 1 else 0
                    while len(deferred) > keep:
                        emit_store(*deferred.pop(0))
                tail_prev = tail_new if tail_new is not None else ztail
            while deferred:
                emit_store(*deferred.pop(0))
    nc.compile()
    return nc


def _get_nc():
    global _CACHED_NC
    if _CACHED_NC is None:
        _CACHED_NC = _build_program()
    return _CACHED_NC


def kernel(x, transfer, mixer_matrix, gain, _trace=False):
    import ml_dtypes
    from concourse.bass_utils import run_bass_kernel_spmd

    x = np.ascontiguousarray(np.asarray(x, np.float32))
    transfer = np.asarray(transfer, np.float32)
    mixer_matrix = np.asarray(mixer_matrix, np.float32)
    gain = np.asarray(gain, np.float32)

    bf = ml_dtypes.bfloat16
    wf4_np = build_wf4().astype(bf)
    wcol_np = build_wcol().astype(np.float32)
    wi4_np = build_wi4().astype(bf)
    wicol_np = (float(gain.ravel()[0]) * build_wcol()).astype(np.float32)
    Ts = build_t_slots(transfer)
    eye = np.eye(128, dtype=np.float32)
    eyeb_np = np.eye(128, dtype=np.float64).astype(bf)

    in_maps = []
    for c in range(8):
        b, dh = c // 2, c % 2
        mixw4 = build_mixw4(
            np.asarray(mixer_matrix, np.float64)[:, dh * DSH:(dh + 1) * DSH]
        ).astype(bf)
        patc = build_pattern(Ts[dh * DSH:(dh + 1) * DSH]).astype(bf)
        in_maps.append({
            "xb": x[b].astype(bf),
            "mixw4": mixw4,
            "wf4": wf4_np,
            "wcol": wcol_np,
            "wi4": wi4_np,
            "wicol": wicol_np,
            "pat": patc,
            "eye": eye,
            "eyeb": eyeb_np,
        })

    nc = _get_nc()
    res = run_bass_kernel_spmd(nc, in_maps, list(range(8)), trace=_trace)
    out = np.zeros((BATCH, CPD, TIME), np.float32)
    for c in range(8):
        b, dh = c // 2, c % 2
        out[b, dh * DSH:(dh + 1) * DSH] = res.results[c]["y"]
    if _trace:
        return out, res
    return out



# revision 30
# speedup vs baseline: 1.0476x; 1.0271x over previous
"""Trainium2 Bass kernel for nn_Block_38517266710836.

reference pipeline: channel mixer -> STFT (hann 2048, hop 1024) -> per-frame
recurrence out[f] = (spec[f] + out[f-1]) * transfer -> iSTFT (hann synthesis)
-> overlap-add -> gain -> tanh.

Sharding: 8 cores, data-parallel over (batch, channel-half): core c handles
batch c//2, mixed channels [32*(c%2), +32). Each core receives its batch's
full 64-channel input (the mixer contracts channels) and writes 32 rows.

Pipelined single-pass program per core: mixer chunks, forward-DFT frame
batches, the DVE recurrence scan, and the inverse/overlap-add phase are
interleaved so the scan and evictions hide under PE matmul work.  Forward
evictions and corner-turn copies run on ScalarE (DVE is reserved for the
scan), weights stream on the gpsimd DMA queue, x/y on the sync queue, and
PSUM pools are shared across phases to fit the 8-bank budget.
"""

import numpy as np

WINDOW = 2048
STEP = 1024
CPD = 64
BATCH = 4
TIME = 65536
FRAMES = 64
NJ = 16              # per-frame time chunks (fwd contraction blocks)
NM = 16              # spectral slot chunks
DSH = 32             # mixed channels per core
GCH = TIME // 128    # 512 global 128-sample chunks
GPAD = GCH + 16      # + zero pad (frame 63 reaches t=66560; extra width so
                     # the forward rhs slice [base, base+2048) stays in-bounds)
FC = 4               # frame chunks for the scan layout
FW = 16              # frames per chunk
CB = 17              # chain block: 1 inject/reset col + 16 frame cols
SPECW = NM * DSH * CB  # 8704 free cols per fc block


def _hann(n):
    return (0.5 - 0.5 * np.cos(2.0 * np.pi * np.arange(n) / n)).astype(np.float64)


def _slot_tables():
    """slot s in [0,2048): s<1024 -> Re[k=s]; s==1024 -> Re[1024] (parked in
    Im[0]'s slot, since Im[0] is identically 0); s>1024 -> Im[k=s-1024]."""
    k_of_slot = np.zeros(2048, np.int64)
    is_im = np.zeros(2048, np.bool_)
    for s in range(2048):
        if s < 1024:
            k_of_slot[s] = s
        elif s == 1024:
            k_of_slot[s] = 1024
        else:
            k_of_slot[s] = s - 1024
            is_im[s] = True
    return k_of_slot, is_im


def build_fwd_weights():
    """[2048 n, 2048 slots]: windowed rfft of one frame, slot layout."""
    n = np.arange(WINDOW, dtype=np.float64)
    w = _hann(WINDOW)
    k_of_slot, is_im = _slot_tables()
    ang = 2.0 * np.pi * np.outer(n, k_of_slot.astype(np.float64)) / WINDOW
    W = np.where(is_im[None, :], -np.sin(ang), np.cos(ang))
    W *= w[:, None]
    return W


def build_inv_weights(gain):
    """[2048 slots, 2048 n]: gain * hann * irfft from slot layout."""
    n = np.arange(WINDOW, dtype=np.float64)
    w = _hann(WINDOW)
    k_of_slot, is_im = _slot_tables()
    ang = 2.0 * np.pi * np.outer(k_of_slot.astype(np.float64), n) / WINDOW
    k = k_of_slot
    re_coef = (2.0 - (k == 0) - (k == 1024))[:, None] / WINDOW * np.cos(ang)
    im_coef = -2.0 / WINDOW * np.sin(ang)
    W = np.where(is_im[:, None], im_coef, re_coef)
    W[1024, :] = np.cos(np.pi * n) / WINDOW
    W *= (gain * w)[None, :]
    return W


def _slot_tables_r4():
    """family-major slot layout: slot' = s*512 + local; family s holds
    k = s, s+4, ... <= 1024(ish), (re, im) interleaved k-major."""
    karr = np.zeros(2048, np.int64)
    isim = np.zeros(2048, np.bool_)
    pos = 0
    for s in range(4):
        for k in range(s, 1025, 4):
            karr[pos] = k; isim[pos] = False; pos += 1
            if k not in (0, 1024):
                karr[pos] = k; isim[pos] = True; pos += 1
    assert pos == 2048
    return karr, isim


_R4_PLANES = {0: [0], 1: [2, 3], 2: [1], 3: [2, 3]}  # m//4 -> plane list


def _build_wfam():
    """family -> list of (plane, [512 q, 512 r]) weight matrices.
    planes: 0=y0, 1=y2, 2=m0, 3=m1."""
    q = np.arange(512, dtype=np.float64)
    karr, isim = _slot_tables_r4()
    fams = {}
    for s in range(4):
        kv = karr[s * 512:(s + 1) * 512].astype(np.float64)
        iv = isim[s * 512:(s + 1) * 512]
        ang = 2.0 * np.pi * np.outer(q, kv) / WINDOW
        c, sn = np.cos(ang), np.sin(ang)
        if s == 0:
            fams[s] = [(0, np.where(iv[None, :], -sn, c))]
        elif s == 2:
            fams[s] = [(1, np.where(iv[None, :], -sn, c))]
        elif s == 1:
            fams[s] = [(2, np.where(iv[None, :], -sn, c)),
                       (3, np.where(iv[None, :], -c, -sn))]
        else:
            fams[s] = [(2, np.where(iv[None, :], -sn, c)),
                       (3, np.where(iv[None, :], c, sn))]
    return fams


def build_wf4():
    """[128, 96*128] SBUF-ready block layout matching the device MM loop:
    for qp, mi: m=2qp+mi -> (s=m//4, subm=m%4): for plane, for qc: block
    = Wfam[s][plane][qc*128:+128, subm*128:+128]."""
    fams = _build_wfam()
    blocks = []
    for qp in range(8):
        for mi in range(2):
            m = qp * 2 + mi
            s, subm = m // 4, m % 4
            for pl, Wm in fams[s]:
                for qc in range(4):
                    blocks.append(Wm[qc * 128:(qc + 1) * 128,
                                     subm * 128:(subm + 1) * 128])
    return np.concatenate(blocks, axis=1)  # [128, 96*128]


def build_wcol():
    """[128, 16] per-partition window scalars: col j*4+qc = w[qc*128+p+512j]."""
    w = _hann(WINDOW)
    out = np.zeros((128, 16), np.float64)
    for j in range(4):
        for qc in range(4):
            out[:, j * 4 + qc] = w[qc * 128 + np.arange(128) + 512 * j]
    return out


def build_t_slots(transfer):
    karr, _ = _slot_tables_r4()
    return np.asarray(transfer, np.float64)[:, karr]  # [ch, 2048]


_INV_PLANES = [(0, False), (1, False), (1, True), (2, False), (3, False), (3, True)]
# zb plane order: 0=zre0, 1=zre1, 2=zim1, 3=zre2, 4=zre3, 5=zim3


def build_wi4():
    """[128, 96*128] inverse z-plane weights; device order:
    for qc in 4: for pl in 6: for subm in 4."""
    karr, isim = _slot_tables_r4()
    q = np.arange(512, dtype=np.float64)
    Vs = []
    for (s, want_im) in _INV_PLANES:
        kv = karr[s * 512:(s + 1) * 512].astype(np.float64)
        iv = isim[s * 512:(s + 1) * 512]
        coef = (2.0 - (kv == 0) - (kv == 1024)) / WINDOW
        ang = 2.0 * np.pi * np.outer(kv, q) / WINDOW
        c, sn = np.cos(ang), np.sin(ang)
        V = coef[:, None] * (np.where(iv[:, None], c, sn) if want_im
                             else np.where(iv[:, None], -sn, c))
        Vs.append(V)  # [512 slot-reals, 512 q]
    blocks = []
    for qc in range(4):
        for V in Vs:
            for subm in range(4):
                blocks.append(V[subm * 128:(subm + 1) * 128,
                               qc * 128:(qc + 1) * 128])
    return np.concatenate(blocks, axis=1)


def build_inv_weights_perm(gain):
    """[2048 slots', 2048 n]: gain * hann * irfft from the r4 slot layout."""
    n = np.arange(WINDOW, dtype=np.float64)
    w = _hann(WINDOW)
    karr, isim = _slot_tables_r4()
    k = karr.astype(np.float64)
    ang = 2.0 * np.pi * np.outer(k, n) / WINDOW
    re_coef = (2.0 - (karr == 0) - (karr == 1024))[:, None] / WINDOW * np.cos(ang)
    im_coef = -2.0 / WINDOW * np.sin(ang)
    W = np.where(isim[:, None], im_coef, re_coef)
    W *= (gain * w)[None, :]
    return W


def build_fwd_weights_perm():
    """effective [2048 n, 2048 slots'] fwd matrix (validation only)."""
    n = np.arange(WINDOW, dtype=np.float64)
    w = _hann(WINDOW)
    karr, isim = _slot_tables_r4()
    ang = 2.0 * np.pi * np.outer(n, karr.astype(np.float64)) / WINDOW
    W = np.where(isim[None, :], -np.sin(ang), np.cos(ang))
    W *= w[:, None]
    return W


def build_pattern(t_slots_core):
    """T-pattern [128, SPECW]: per (m,d) chain block of CB cols:
    col 0 = 0 (reset: state=(x+state)*0), cols 1..16 = T[slot(m,kf), d]
    for the scan state=(spec+state)*pat; the carry between fc blocks is
    pre-added into the next block's frame-0 column."""
    pat = np.zeros((128, SPECW), np.float64)
    for m in range(NM):
        for d in range(DSH):
            base = (m * DSH + d) * CB
            pat[:, base + 1: base + CB] = \
                t_slots_core[d, m * 128:(m + 1) * 128][:, None]
    return pat


def build_mixw4(mix_half):
    """[128, 64] block-diag lhsT: rows (q,c) q in 2, cols (q',d):
    M[c, d] if q==q' else 0."""
    out = np.zeros((128, 64), np.float64)
    out[0:64, 0:32] = mix_half
    out[64:128, 32:64] = mix_half
    return out


def emulate(x, transfer, mixer_matrix, gain, wdtype=np.float32):
    """Numpy emulation of the device math (offline validation)."""
    b, c, t = x.shape
    Wf = build_fwd_weights_perm().astype(wdtype).astype(np.float64)
    Wi = build_inv_weights_perm(float(np.asarray(gain).ravel()[0])).astype(wdtype).astype(np.float64)
    Ts = build_t_slots(transfer)
    y = np.einsum('bct,cd->bdt', np.asarray(x, np.float64),
                  np.asarray(mixer_matrix, np.float64))
    yp = np.pad(y, ((0, 0), (0, 0), (0, STEP)))
    out = np.zeros((b, c, t), np.float64)
    for bi in range(b):
        frames = np.stack([yp[bi, :, f * STEP: f * STEP + WINDOW]
                           for f in range(FRAMES)], 1)
        spec = frames.astype(wdtype).astype(np.float64) @ Wf
        st = np.zeros((c, 2048))
        outs = np.zeros_like(spec)
        for f in range(FRAMES):
            st = (spec[:, f].astype(wdtype).astype(np.float64) + st) * Ts
            outs[:, f] = st
        aud = outs.astype(wdtype).astype(np.float64) @ Wi
        acc = np.zeros((c, t + STEP))
        for f in range(FRAMES):
            acc[:, f * STEP: f * STEP + WINDOW] += aud[:, f]
        out[bi] = np.tanh(acc[:, :t])
    return out.astype(np.float32)


# ---------------------------------------------------------------------------
# Device program
# ---------------------------------------------------------------------------

_CACHED_NC = None


def _build_program():
    import concourse.bacc as bacc
    import concourse.mybir as mybir
    from concourse import tile
    from contextlib import ExitStack

    f32 = mybir.dt.float32
    bf16 = mybir.dt.bfloat16
    Alu = mybir.AluOpType

    nc = bacc.Bacc("TRN2", target_bir_lowering=False, debug=False, num_devices=8)
    xb = nc.dram_tensor("xb", [CPD, TIME], bf16, kind="ExternalInput").ap()
    mixwd = nc.dram_tensor("mixw4", [128, 64], bf16, kind="ExternalInput").ap()
    eyebd = nc.dram_tensor("eyeb", [128, 128], bf16, kind="ExternalInput").ap()
    wf4 = nc.dram_tensor("wf4", [128, 96 * 128], bf16, kind="ExternalInput").ap()
    wcold = nc.dram_tensor("wcol", [128, 16], f32, kind="ExternalInput").ap()
    wi4d = nc.dram_tensor("wi4", [128, 96 * 128], bf16, kind="ExternalInput").ap()
    wicold = nc.dram_tensor("wicol", [128, 16], f32, kind="ExternalInput").ap()
    patd = nc.dram_tensor("pat", [128, SPECW], bf16, kind="ExternalInput").ap()
    eyed = nc.dram_tensor("eye", [128, 128], f32, kind="ExternalInput").ap()
    yout = nc.dram_tensor("y", [DSH, TIME], f32, kind="ExternalOutput").ap()

    XCH = 4096           # x streamed in 4096-sample chunks
    NXC = TIME // XCH    # 16
    # scan split: DVE takes m-blocks 0..8, Pool 9..15
    SCAN_DVE = 9 * DSH * CB   # 4896 cols
    # x view [nx, q, c, h, f]: t = nx*4096 + h*2048 + q*1024 + f
    xbv = xb.rearrange("c (nx h q f) -> nx q c h f", nx=NXC, h=2, q=2)

    with tile.TileContext(nc) as tc, ExitStack() as ctx:
        persist = ctx.enter_context(tc.tile_pool(name="persist", bufs=1))
        spec = persist.tile([128, FC * SPECW], bf16, tag="spec")
        pat = persist.tile([128, SPECW], bf16, tag="pat")
        mx4 = persist.tile([128, 64], bf16, tag="mx4")
        eyeb = persist.tile([128, 128], bf16, tag="eyeb")
        eye = persist.tile([128, 128], f32, tag="eye")
        wcol = persist.tile([128, 16], f32, tag="wcol")

        xin = ctx.enter_context(tc.tile_pool(name="xin", bufs=3))
        ymp = ctx.enter_context(tc.tile_pool(name="ymp", bufs=3))



        # reset cols (col 0 of every chain) must not contain NaN garbage:
        # (garbage + state) * 0 is 0 only for finite garbage
        nc.vector.memset(
            spec[:].rearrange("p (fcmd c) -> p fcmd c", c=CB)[:, :, 0:1], 0.0)

        # Pool/gpsimd cannot access PSUM (walrus birverifier) — psum
        # evictions go on Act/DVE only
        _TURN_ENG = [nc.scalar, nc.vector]

        def mixer_chunk(xc):
            # xt partitions (q,c), free (h,f): t = xc*4096 + h*2048 + q*1024 + f
            xt = xin.tile([128, 2048], bf16, tag="x", name=f"x{xc}")
            for q in range(2):
                nc.sync.dma_start(
                    out=xt[q * 64:(q + 1) * 64, :].rearrange(
                        "c (h f) -> c h f", h=2),
                    in_=xbv[xc, q])
            pm = ppM.tile([128, 1024], f32, tag="pp", name=f"mix{xc}")
            for h in range(2):
                for hf in range(2):  # psum-bank-sized output halves
                    nc.tensor.matmul(
                        pm[h * 64:(h + 1) * 64,
                           hf * 512:(hf + 1) * 512],
                        mx4[:],
                        xt[:, h * 1024 + hf * 512: h * 1024 + (hf + 1) * 512],
                        start=True, stop=True)
            # ym[(h,q,d), f]
            ym = ymp.tile([128, 1024], bf16, tag="ym", name=f"ym{xc}")
            nc.scalar.copy(ym[:], pm[:])
            # corner turn: 8 PE transposes, batched 4 per psum tile, then one
            # [128,512] strided eviction per tile spread across engines
            for j in range(2):
                pt = ppB.tile([128, 512], bf16, tag="pt", name=f"turn{xc}_{j}")
                for s4 in range(4):
                    s = j * 4 + s4
                    nc.tensor.transpose(
                        pt[:, s4 * 128:(s4 + 1) * 128],
                        ym[:, s * 128:(s + 1) * 128],
                        eyeb[:])
                # pt[tf, (s4,h,q,d)] -> a_t[tf, (g,d)], g = xc*32+h*16+q*8+s
                dst = a_t[:][:, (xc * 32) * DSH:(xc * 32 + 32) * DSH] \
                    .rearrange("p (hq blk d) -> p hq blk d", hq=4, blk=8)[
                        :, :, j * 4:(j + 1) * 4, :]
                psrc = pt[:].rearrange("p (s4 hq d) -> p hq s4 d", s4=4, hq=4)
                eng = _TURN_ENG[(2 * xc + j) % 2]
                if eng is nc.scalar:
                    eng.copy(dst, psrc)
                else:
                    eng.tensor_copy(dst, psrc)

        def precombine(b, wf_t, ztp, xwp, tmpp):
            """butterfly planes for frame batch b: zt cols (plane, qc, f, d);
            planes 0=y0, 1=y2, 2=m0, 3=m1.  Window applied via per-partition
            tensor_scalar on GpSimd; adds on DVE."""
            zt = ztp.tile([128, 4 * 4 * 512], bf16, tag="zt", name=f"zt{b}")
            for qc in range(4):
                xw = xwp.tile([128, 2048], bf16, tag="xw", name=f"xw{b}_{qc}")
                for j in range(4):
                    base = (128 * b + qc + 4 * j) * DSH
                    view = a_t[:][:, base: base + 4096] \
                        .rearrange("p (f q) -> p f q", f=16)[:, :, :DSH]
                    nc.gpsimd.tensor_scalar_mul(
                        xw[:, j * 512:(j + 1) * 512]
                        .rearrange("p (f d) -> p f d", f=16),
                        view, wcol[:, j * 4 + qc: j * 4 + qc + 1])
                tmp = tmpp.tile([128, 1024], bf16, tag="tmp", name=f"tm{b}_{qc}")
                nc.vector.tensor_add(tmp[:, :512], xw[:, 0:512], xw[:, 1024:1536])
                nc.vector.tensor_add(tmp[:, 512:], xw[:, 512:1024], xw[:, 1536:2048])
                z = lambda pl: zt[:, (pl * 4 + qc) * 512:(pl * 4 + qc + 1) * 512]
                nc.vector.tensor_sub(z(2), xw[:, 0:512], xw[:, 1024:1536])
                nc.vector.tensor_sub(z(3), xw[:, 512:1024], xw[:, 1536:2048])
                nc.vector.tensor_add(z(0), tmp[:, :512], tmp[:, 512:])
                nc.vector.tensor_sub(z(1), tmp[:, :512], tmp[:, 512:])
            return zt

        def fwd_batch(f16, wf_t, zt):
            # radix-4 forward: per m-block, accumulate plane x qc matmuls
            fc = f16
            blk = [0]
            for m in range(16):
                ps = sp.tile([128, 512], f32, tag="sm", name=f"sm{f16}_{m}")
                s = m // 4
                planes = _R4_PLANES[s]
                out_ap = ps[:].rearrange("p (d f) -> p f d", f=16)
                nmm = len(planes) * 4
                i = 0
                for pl in planes:
                    for qc in range(4):
                        rhs = zt[:, (pl * 4 + qc) * 512:(pl * 4 + qc + 1) * 512] \
                            .rearrange("p (f d) -> p f d", f=16)
                        nc.tensor.matmul(
                            out_ap,
                            wf_t[:, blk[0] * 128:(blk[0] + 1) * 128],
                            rhs,
                            start=(i == 0), stop=(i == nmm - 1))
                        blk[0] += 1
                        i += 1
                # per-m eviction (ScalarE; DVE is scanning)
                src_ = ps[:].rearrange("p (d f) -> p d f", f=16)
                doff = fc * SPECW + m * DSH * CB
                dst = spec[:][:, doff: doff + DSH * CB] \
                    .rearrange("p (d c) -> p d c", c=CB)[:, :, 1: 1 + FW]
                nc.scalar.copy(dst, src_)

        def scan_block(fc):
            # direct recurrence out[f] = (spec[f] + out[f-1]) * T via
            # state = (data0 + state) * data1, split DVE (m 0..8) / Pool
            # (m 9..15). pat col0 = 1 so col0 passes the injected carry.
            base = fc * SPECW
            nc.vector.tensor_tensor_scan(
                spec[:, base:base + SCAN_DVE],
                spec[:, base:base + SCAN_DVE],
                pat[:, 0:SCAN_DVE],
                0.0, Alu.add, Alu.mult)
            nc.vector.tensor_tensor_scan(
                spec[:, base + SCAN_DVE:base + SPECW],
                spec[:, base + SCAN_DVE:base + SPECW],
                pat[:, SCAN_DVE:SPECW],
                0.0, Alu.add, Alu.mult)

        def inject_block(fc):
            # pre-add the carry (out[15] of block fc) into block fc+1's
            # frame-0 column; must run AFTER fwd(fc+1)'s evictions have
            # written the raw col (they would clobber it otherwise) and
            # before scan(fc+1)
            base = fc * SPECW
            nmd_dve = SCAN_DVE // CB
            srcv = spec[:][:, base: base + SPECW] \
                .rearrange("p (md c) -> p md c", c=CB)[:, :, CB - 1: CB]
            dstv = spec[:][:, base + SPECW: base + 2 * SPECW] \
                .rearrange("p (md c) -> p md c", c=CB)[:, :, 1:2]
            nc.vector.tensor_add(dstv[:, :nmd_dve], dstv[:, :nmd_dve],
                                 srcv[:, :nmd_dve])
            nc.vector.tensor_add(dstv[:, nmd_dve:], dstv[:, nmd_dve:],
                                 srcv[:, nmd_dve:])

        # ================= phase F (+ scan), pipelined =================
        with ExitStack() as ctxF:
            wp = ctxF.enter_context(tc.tile_pool(name="wfp", bufs=1))
            # phase F PSUM: mixer pm [128,1024] x2 = 4 banks, fwd sp x3 = 3
            wf_t = wp.tile([128, 96 * 128], bf16, tag="wf")
            ztp = ctxF.enter_context(tc.tile_pool(name="ztp", bufs=2))
            xwp = ctxF.enter_context(tc.tile_pool(name="xwp", bufs=2))
            tmpp = ctxF.enter_context(tc.tile_pool(name="tmpp", bufs=2))
            sp = ctxF.enter_context(tc.tile_pool(name="sp", bufs=2, space="PSUM"))
            # mixer-side tiles freed after precombine(3) to make room for wi_t
            ctxA = ExitStack()
            pa = ctxA.enter_context(tc.tile_pool(name="pa", bufs=1))
            ppM = ctxA.enter_context(tc.tile_pool(name="ppM", bufs=2, space="PSUM"))
            ppB = ctxA.enter_context(tc.tile_pool(name="ppB", bufs=2, space="PSUM"))
            xin = ctxA.enter_context(tc.tile_pool(name="xin", bufs=3))
            ymp = ctxA.enter_context(tc.tile_pool(name="ymp", bufs=3))
            a_t = pa.tile([128, GPAD * DSH], bf16, tag="a")
            nc.vector.memset(a_t[:, GCH * DSH:], 0.0)

            nc.sync.dma_start(out=mx4[:], in_=mixwd[:])
            nc.sync.dma_start(out=eyeb[:], in_=eyebd[:])
            mixer_chunk(0)
            nc.sync.dma_start(out=wcol[:], in_=wcold[:])
            # weights on the Act queue (x streams on sync unimpeded), in
            # slices so x/weight transfers interleave on the DMA engines
            for s in range(8):
                nc.scalar.dma_start(out=wf_t[:, s * 1536:(s + 1) * 1536],
                                    in_=wf4[:, s * 1536:(s + 1) * 1536])
            nc.sync.dma_start(out=eye[:], in_=eyed[:])
            for xc in range(1, 5):
                mixer_chunk(xc)
            nc.scalar.dma_start(out=pat[:, :SPECW // 2], in_=patd[:, :SPECW // 2])
            nc.scalar.dma_start(out=pat[:, SPECW // 2:], in_=patd[:, SPECW // 2:])
            zt0 = precombine(0, wf_t, ztp, xwp, tmpp)
            for xc in range(5, 9):
                mixer_chunk(xc)
            fwd_batch(0, wf_t, zt0)
            zt1 = precombine(1, wf_t, ztp, xwp, tmpp)
            scan_block(0)
            for xc in range(9, 13):
                mixer_chunk(xc)
            fwd_batch(1, wf_t, zt1)
            inject_block(0)
            zt2 = precombine(2, wf_t, ztp, xwp, tmpp)
            scan_block(1)
            for xc in range(13, NXC):
                mixer_chunk(xc)
            fwd_batch(2, wf_t, zt2)
            inject_block(1)
            zt3 = precombine(3, wf_t, ztp, xwp, tmpp)
            ctxA.close()
            scan_block(2)
            fwd_batch(3, wf_t, zt3)
            inject_block(2)
            scan_block(3)

        # ================= phase I (radix-4 inverse) =================
        with ExitStack() as ctxI:
            wp2 = ctxI.enter_context(tc.tile_pool(name="wip2", bufs=1))
            # phase I PSUM: 6 plane psums + 2 emit-transpose psums = 8 banks
            zpl = ctxI.enter_context(tc.tile_pool(name="zpl", bufs=6, space="PSUM"))
            ppT = ctxI.enter_context(tc.tile_pool(name="ppT", bufs=2, space="PSUM"))
            wi_t = wp2.tile([128, 96 * 128], bf16, tag="wi")
            # sliced in consumption (qc) order: the first slice's ~2.2us
            # transfer is the only phase-I startup exposure
            for s in range(4):
                nc.scalar.dma_start(out=wi_t[:, s * 3072:(s + 1) * 3072],
                                    in_=wi4d[:, s * 3072:(s + 1) * 3072])
            wicol = wp2.tile([128, 16], f32, tag="wicol")
            nc.sync.dma_start(out=wicol[:], in_=wicold[:])
            ztail = wp2.tile([128, 2 * 4 * DSH], bf16, tag="ztail")
            nc.vector.memset(ztail[:], 0.0)

            tout = ctxI.enter_context(tc.tile_pool(name="tout", bufs=6))
            stg = ctxI.enter_context(tc.tile_pool(name="stg", bufs=3))
            zbp = ctxI.enter_context(tc.tile_pool(name="zbp", bufs=3))
            efp = ctxI.enter_context(tc.tile_pool(name="efp", bufs=3))
            aqp = ctxI.enter_context(tc.tile_pool(name="aqp", bufs=3))
            ywp = ctxI.enter_context(tc.tile_pool(name="ywp", bufs=3))
            ohp = ctxI.enter_context(tc.tile_pool(name="ohp", bufs=3))
            tailp = ctxI.enter_context(tc.tile_pool(name="tailp", bufs=3))

            yv = yout.rearrange("d (a4 fl t) -> fl d a4 t", fl=4, t=1024)

            st2_cur = {}

            def emit_store(tt, fc, qc, h):
                p4 = ppT.tile([128, 512], f32, tag="pp",
                              name=f"t4_{fc}_{qc}_{h}")
                for r2 in range(4):
                    nc.tensor.transpose(
                        p4[:, r2 * 128:(r2 + 1) * 128],
                        tt[:, r2 * 128:(r2 + 1) * 128],
                        eye[:])
                # stage 4 qc-groups into one [128, (r2, qc, t)] tile so the
                # y DMAs coarsen 4x (32 total instead of 128): the per-DMA
                # 650ns queue-issue cost was the phase-I drain bottleneck
                if (fc, h) not in st2_cur:
                    st2_cur[(fc, h)] = stg.tile([128, 2048], f32, tag="stg",
                                                name=f"st{fc}_{h}")
                st2 = st2_cur[(fc, h)]
                nc.scalar.copy(
                    st2[:].rearrange("p (r2 q t) -> p r2 q t",
                                     r2=4, q=4)[:, :, qc, :],
                    p4[:].rearrange("p (r2 t) -> p r2 t", r2=4))
                # all y DMAs on sync (head-parked DMAs must not block Act
                # compute); for the last fc, flush qc-pair halves so the
                # final drain overlaps the remaining compute
                if fc == FC - 1 and qc == 1:
                    for r2 in range(4):
                        dst = yv[:, :, 4 * fc + r2, 512 * h: 512 * h + 256]
                        nc.sync.dma_start(
                            out=dst,
                            in_=st2[:].rearrange(
                                "p (r2 q t) -> p r2 q t", r2=4, q=4)[
                                :, r2, 0:2, :])
                elif qc == 3:
                    if fc == FC - 1:
                        for r2 in range(4):
                            dst = yv[:, :, 4 * fc + r2,
                                     512 * h + 256: 512 * h + 512]
                            nc.sync.dma_start(
                                out=dst,
                                in_=st2[:].rearrange(
                                    "p (r2 q t) -> p r2 q t", r2=4, q=4)[
                                    :, r2, 2:4, :])
                    else:
                        for r2 in range(4):
                            dst = yv[:, :, 4 * fc + r2, 512 * h: 512 * h + 512]
                            eng = nc.sync if r2 < 2 else nc.scalar
                            eng.dma_start(
                                out=dst, in_=st2[:, r2 * 512:(r2 + 1) * 512])
                    del st2_cur[(fc, h)]

            # Pool cannot read PSUM; fc0 avoids DVE (still draining scan(3))
            _ZB_ENG0 = [nc.scalar] * 6
            _ZB_ENG = [nc.scalar, nc.vector, nc.scalar,
                       nc.vector, nc.scalar, nc.scalar]
            deferred = []

            def postprocess(zb, fc, qc, tail_prev, tail_new):
                # butterflies (DVE, bf16): e,f,gg,h then quarters a0..a3
                ef = efp.tile([128, 4 * 512], bf16, tag="ef",
                              name=f"ef{fc}_{qc}")
                z = lambda pl: zb[:, pl * 512:(pl + 1) * 512]
                nc.vector.tensor_add(ef[:, 0 * 512:1 * 512], z(0), z(3))   # e
                nc.vector.tensor_sub(ef[:, 1 * 512:2 * 512], z(0), z(3))   # f
                nc.vector.tensor_add(ef[:, 2 * 512:3 * 512], z(1), z(4))   # gg
                nc.vector.tensor_sub(ef[:, 3 * 512:4 * 512], z(5), z(2))   # h
                aq = aqp.tile([128, 4 * 512], bf16, tag="aq",
                              name=f"aq{fc}_{qc}")
                E, F_, G, H = (ef[:, i * 512:(i + 1) * 512] for i in range(4))
                nc.vector.tensor_add(aq[:, 0 * 512:1 * 512], E, G)   # a0
                nc.vector.tensor_add(aq[:, 1 * 512:2 * 512], F_, H)  # a1
                nc.vector.tensor_sub(aq[:, 2 * 512:3 * 512], E, G)   # a2
                nc.vector.tensor_sub(aq[:, 3 * 512:4 * 512], F_, H)  # a3
                # save pre-window tail quarters (a2,a3 of frame 15)
                if tail_new is not None:
                    for j2 in range(2):
                        nc.vector.tensor_copy(
                            tail_new[:, (j2 * 4 + qc) * DSH:
                                     (j2 * 4 + qc + 1) * DSH],
                            aq[:, (2 + j2) * 512 + 15 * DSH:
                               (2 + j2) * 512 + 16 * DSH])
                # window (GpSimd, per-partition scalars) + OLA + tanh
                for h in range(2):
                    yw = ywp.tile([128, 1024], bf16, tag="yw",
                                  name=f"yw{fc}_{qc}_{h}")
                    nc.gpsimd.tensor_scalar_mul(
                        yw[:, :512], aq[:, h * 512:(h + 1) * 512],
                        wicol[:, h * 4 + qc: h * 4 + qc + 1])
                    nc.gpsimd.tensor_scalar_mul(
                        yw[:, 512:], aq[:, (h + 2) * 512:(h + 3) * 512],
                        wicol[:, (h + 2) * 4 + qc: (h + 2) * 4 + qc + 1])
                    # windowed tail quarter for frame 0 of this batch
                    wt = ywp.tile([128, DSH], bf16, tag="wt",
                                  name=f"wt{fc}_{qc}_{h}")
                    nc.gpsimd.tensor_scalar_mul(
                        wt[:], tail_prev[:, (h * 4 + qc) * DSH:
                                         (h * 4 + qc + 1) * DSH],
                        wicol[:, (h + 2) * 4 + qc: (h + 2) * 4 + qc + 1])
                    oh = ohp.tile([128, 512], bf16, tag="oh",
                                  name=f"oh{fc}_{qc}_{h}")
                    nc.vector.tensor_add(
                        oh[:, DSH:], yw[:, DSH:512], yw[:, 512:1024 - DSH])
                    nc.vector.tensor_add(oh[:, :DSH], yw[:, :DSH], wt[:])
                    tt = tout.tile([128, 512], f32, tag="to",
                                   name=f"to{fc}_{qc}_{h}")
                    nc.scalar.activation(
                        tt[:], oh[:], mybir.ActivationFunctionType.Tanh)
                    deferred.append((tt, fc, qc, h))
                keep = 4 if fc < FC - 1 else 0
                while len(deferred) > keep:
                    emit_store(*deferred.pop(0))

            # software-pipelined by one group: each group's postprocess is
            # emitted after the NEXT group's matmuls+evictions, so cross-
            # engine waits in the butterfly chain never block the in-order
            # queues' eviction work (which would stall PE on psum banks)
            pending = None
            tail_prev = ztail
            for fc in range(FC):
                tail_new = tailp.tile([128, 2 * 4 * DSH], bf16, tag="tail",
                                      name=f"tail{fc}") if fc < FC - 1 else None
                for qc in range(4):
                    # 6 z-plane transforms: contraction over family slot-reals
                    zb = zbp.tile([128, 6 * 512], bf16, tag="zb",
                                  name=f"zb{fc}_{qc}")
                    for pl in range(6):
                        s = _INV_PLANES[pl][0]
                        ps = zpl.tile([128, 512], f32, tag="pp",
                                      name=f"zp{fc}_{qc}_{pl}")
                        out_ap = ps[:].rearrange("p (f d) -> p d f", f=FW)
                        for subm in range(4):
                            m = 4 * s + subm
                            base = fc * SPECW + m * DSH * CB
                            rhs = spec[:][:, base: base + DSH * CB] \
                                .rearrange("p (d c) -> p d c", c=CB)[:, :, 1: 1 + FW]
                            blk = (qc * 6 + pl) * 4 + subm
                            nc.tensor.matmul(
                                out_ap,
                                wi_t[:, blk * 128:(blk + 1) * 128],
                                rhs, start=(subm == 0), stop=(subm == 3))
                        eng = (_ZB_ENG0 if fc == 0 else _ZB_ENG)[pl]
                        if eng is nc.scalar:
                            eng.copy(zb[:, pl * 512:(pl + 1) * 512], ps[:])
                        else:
                            eng.tensor_copy(
                                zb[:, pl * 512:(pl + 1) * 512], ps[:])
                    if pending is not None:
                        postprocess(*pending)
                    pending = (zb, fc, qc, tail_prev, tail_new)
                tail_prev = tail_new if tail_new is not None else ztail
            postprocess(*pending)
            while deferred:
                emit_store(*deferred.pop(0))
    nc.compile()
    return nc


def _get_nc():
    global _CACHED_NC
    if _CACHED_NC is None:
        _CACHED_NC = _build_program()
    return _CACHED_NC


def kernel(x, transfer, mixer_matrix, gain, _trace=False):
    import ml_dtypes
    from concourse.bass_utils import run_bass_kernel_spmd

    x = np.ascontiguousarray(np.asarray(x, np.float32))
    transfer = np.asarray(transfer, np.float32)
    mixer_matrix = np.asarray(mixer_matrix, np.float32)
    gain = np.asarray(gain, np.float32)

    bf = ml_dtypes.bfloat16
    wf4_np = build_wf4().astype(bf)
    wcol_np = build_wcol().astype(np.float32)
    wi4_np = build_wi4().astype(bf)
    wicol_np = (float(gain.ravel()[0]) * build_wcol()).astype(np.float32)
    Ts = build_t_slots(transfer)
    eye = np.eye(128, dtype=np.float32)
    eyeb_np = np.eye(128, dtype=np.float64).astype(bf)

    in_maps = []
    for c in range(8):
        b, dh = c // 2, c % 2
        mixw4 = build_mixw4(
            np.asarray(mixer_matrix, np.float64)[:, dh * DSH:(dh + 1) * DSH]
        ).astype(bf)
        patc = build_pattern(Ts[dh * DSH:(dh + 1) * DSH]).astype(bf)
        in_maps.append({
            "xb": x[b].astype(bf),
            "mixw4": mixw4,
            "wf4": wf4_np,
            "wcol": wcol_np,
            "wi4": wi4_np,
            "wicol": wicol_np,
            "pat": patc,
            "eye": eye,
            "eyeb": eyeb_np,
        })

    nc = _get_nc()
    res = run_bass_kernel_spmd(nc, in_maps, list(range(8)), trace=_trace)
    out = np.zeros((BATCH, CPD, TIME), np.float32)
    for c in range(8):
        b, dh = c // 2, c % 2
        out[b, dh * DSH:(dh + 1) * DSH] = res.results[c]["y"]
    if _trace:
        return out, res
    return out

